# revision 32
# baseline (speedup 1.0000x reference)
"""ASGCN unit kernel for 8 Trainium2 NeuronCores (data-parallel over batch).

Contract: kernel(**inputs) takes the FULL unsharded inputs and returns the
FULL [128, 256] float32 output. Batch is sharded 16 samples/core across 8
cores; all parameters are replicated.

v2 design (evolved from the fp8 baseline after trace analysis):
  - position_weight, degree norm and fp8 scales are folded on host into the
    transposed adjacency (adjTw = adjT * 4096 * w[t] * dinv[s]) exactly as
    before; samples are sorted by n = ceil(text_len/128) into per-core slots
    sharing one slot->n pattern (SPMD) so matmuls skip structurally-zero
    128-chunks.
  - ALL inputs are shipped as a few large per-core packed DRAM blobs and
    loaded with ~18 big DMAs on the sync queue (the old per-sample DMA
    scheme kept the sync engine ~90% busy generating descriptors).
  - attention is restructured to be (almost) all-tensor:
      * logits computed TRANSPOSED: ps_lT[t,1] per 128-chunk via 8 tiny
        matmuls (lhsT = bf16 xT chunk, rhs = xs column),
      * exp on [128,4] (scalar engine, 128-partition utilization) with the
        per-partition accumulator collected into sumP[:, b],
      * weighted sum as 4 N=256 bf16 matmuls against a NORMAL-layout bf16
        copy of text_out (new input), giving the output row [1,256] in PSUM,
      * sum(exp) for all 16 samples reduced with ONE final f32 matmul
        (ones.T @ sumP).
    This removes the p-broadcast matmul, the [128,512] scalar copy and the
    two [128,512] vector accumulate-STTs per sample of the old design.
  - GCN layer 2 (window) runs fp8 for the adjacency contraction then bf16
    for the W2 matmul (better accuracy than the old all-fp8 path).
  - PSUM->SBUF epilogues are spread across scalar/vector; gpsimd (no PSUM
    port) takes the SBUF-only relu+cast work.
"""

import sys

if "/opt/trn_rl_repo" not in sys.path:
    sys.path.insert(0, "/opt/trn_rl_repo")

import numpy as np
import ml_dtypes

B, S, D, WIN = 128, 512, 256, 8
NCORES = 8
BPC = B // NCORES  # samples per core
BF = ml_dtypes.bfloat16
F8 = ml_dtypes.float8_e4m3  # TRN fp8e4: max +-240

_nc_cache = {}
USE_BIAS_MM = True
USE_FUSED_P3B = True


def _offsets(n_slots):
    """Per-slot element offsets (per partition) into the packed blobs."""
    axt_off, adj_off, awm_off = [0], [0], [0]
    for n in n_slots:
        axt_off.append(axt_off[-1] + 2 * 128 * n)      # [128, 2, 128n] fp8
        adj_off.append(adj_off[-1] + n * 128 * n)      # [128, n, 128n] fp8
        awm_off.append(awm_off[-1] + n * WIN)          # [128, n, WIN] fp8
    return axt_off, adj_off, awm_off


def _build_nc(bpc, n_slots):
    from contextlib import ExitStack

    import concourse.bass as bass
    import concourse.tile as tile
    from concourse import bacc, mybir

    dt = mybir.dt
    f32, bf16, f8 = dt.float32, dt.bfloat16, dt.float8e4
    AF = mybir.ActivationFunctionType
    OP = mybir.AluOpType
    DR = mybir.MatmulPerfMode.DoubleRow
    ts = bass.ts

    axt_off, adj_off, awm_off = _offsets(n_slots)

    nc = bacc.Bacc("TRN2", target_bir_lowering=False, debug=False,
                   num_devices=NCORES)

    # --- DRAM parameters: packed per-core blobs ---
    axt8_d = nc.declare_dram_parameter("axt8", [128, axt_off[-1]], f8,
                                       isOutput=False)
    adj8_d = nc.declare_dram_parameter("adj8", [128, adj_off[-1]], f8,
                                       isOutput=False)
    awm_d = nc.declare_dram_parameter("awm", [128, awm_off[-1]], f8,
                                      isOutput=False)
    negm_d = nc.declare_dram_parameter("negm", [1, bpc * WIN], bf16,
                                       isOutput=False)
    axtb_d = nc.declare_dram_parameter("axtb", [128, bpc * 2 * S], bf16,
                                       isOutput=False)
    xnb_d = nc.declare_dram_parameter("xnb", [128, bpc * 2 * S], bf16,
                                      isOutput=False)
    W1_d = nc.declare_dram_parameter("W1s8", [128, 2, D], f8, isOutput=False)
    W2_d = nc.declare_dram_parameter("W2b", [128, 2, D], bf16, isOutput=False)
    b1_d = nc.declare_dram_parameter("b1r16", [1, 2, D], bf16, isOutput=False)
    b2_d = nc.declare_dram_parameter("b2r", [1, 2, 128], bf16, isOutput=False)
    b1B_d = nc.declare_dram_parameter("b1B8", [128, 2, D], bf16, isOutput=False)
    b2c_d = nc.declare_dram_parameter("b2col", [128, 2], f32, isOutput=False)
    mw_d = nc.declare_dram_parameter("mw", [128, bpc * WIN], bf16, isOutput=False)
    outR_d = nc.declare_dram_parameter("outR", [1, bpc * D], f32,
                                       isOutput=True)
    sume_d = nc.declare_dram_parameter("sume", [1, bpc], f32, isOutput=True)

    LAG_P2, LAG_P3, LAG_P4, LAG_WS = 1, 2, 3, 3
    NSTEP = bpc + LAG_WS

    with tile.TileContext(nc) as tc, ExitStack() as ctx:
        const = ctx.enter_context(tc.tile_pool(name="const", bufs=1))
        pmid = ctx.enter_context(tc.tile_pool(name="pmid", bufs=6))
        psmall = ctx.enter_context(tc.tile_pool(name="psmall", bufs=8))
        pstage = ctx.enter_context(tc.tile_pool(name="pstage", bufs=1))
        psH = ctx.enter_context(tc.tile_pool(name="psH", bufs=2, space="PSUM"))
        psG = ctx.enter_context(tc.tile_pool(name="psG", bufs=2, space="PSUM"))
        psS = ctx.enter_context(tc.tile_pool(name="psS", bufs=2, space="PSUM"))
        # logits column [:, 0:4] and the ws output row [0:1, 4:260] share one
        # bank-sized tile (they are serially dependent through exp anyway)
        psLO = ctx.enter_context(tc.tile_pool(name="psLO", bufs=2,
                                              space="PSUM"))

        # ---- input SBUF blobs + the DMA schedule (sync queue only) ----
        W1s8 = const.tile([128, 2, D], f8, tag="W1s8")
        nc.sync.dma_start(W1s8[:], W1_d[:])

        # chunked blobs: independent tiles so readers only wait their chunk;
        # leading chunks are small so the pipeline starts ASAP
        AXT_CH = [(0, 2), (2, 6), (8, 8)]
        ADJ_CH = [(0, 2), (2, 2), (4, 4), (8, 4), (12, 4)]
        ABC, XNC = 4, 4  # slots per chunk
        axt_t, adj_t, axtb_t, xnb_t = {}, {}, {}, {}

        def dma_axt(c0, cnt):
            e0, e1 = axt_off[c0], axt_off[min(c0 + cnt, bpc)]
            t = const.tile([128, e1 - e0], f8, name=f"axt{c0}", tag=f"axt{c0}")
            nc.sync.dma_start(t[:], axt8_d[:, e0:e1])
            for b in range(c0, min(c0 + cnt, bpc)):
                axt_t[b] = (t, axt_off[b] - e0)

        def dma_adj(c0, cnt):
            e0, e1 = adj_off[c0], adj_off[min(c0 + cnt, bpc)]
            t = const.tile([128, e1 - e0], f8, name=f"adj{c0}", tag=f"adj{c0}")
            nc.sync.dma_start(t[:], adj8_d[:, e0:e1])
            for b in range(c0, min(c0 + cnt, bpc)):
                adj_t[b] = (t, adj_off[b] - e0)

        def dma_axtb(c0):
            e0, e1 = c0 * 2 * S, min(c0 + ABC, bpc) * 2 * S
            t = const.tile([128, e1 - e0], bf16, name=f"axb{c0}",
                           tag=f"axb{c0}")
            nc.sync.dma_start(t[:], axtb_d[:, e0:e1])
            for b in range(c0, min(c0 + ABC, bpc)):
                axtb_t[b] = (t, (b - c0) * 2 * S)

        def dma_xnb(c0):
            e0, e1 = c0 * 2 * S, min(c0 + XNC, bpc) * 2 * S
            t = const.tile([128, e1 - e0], bf16, name=f"xnb{c0}",
                           tag=f"xnb{c0}")
            nc.sync.dma_start(t[:], xnb_d[:, e0:e1])
            for b in range(c0, min(c0 + XNC, bpc)):
                xnb_t[b] = (t, (b - c0) * 2 * S)

        dma_axt(*AXT_CH[0])
        dma_adj(*ADJ_CH[0])
        b1r16 = const.tile([1, 2, D], bf16, tag="b1r16")
        nc.sync.dma_start(b1r16[:], b1_d[:])
        dma_axt(*AXT_CH[1])
        dma_adj(*ADJ_CH[1])
        W2b = const.tile([128, 2, D], bf16, tag="W2b")
        nc.sync.dma_start(W2b[:], W2_d[:])
        b2r = const.tile([1, 2, 128], bf16, tag="b2r")
        nc.sync.dma_start(b2r[:], b2_d[:])
        awm = const.tile([128, awm_off[-1]], f8, tag="awm")
        nc.sync.dma_start(awm[:], awm_d[:])
        negm = const.tile([1, bpc * WIN], bf16, tag="negm")
        nc.sync.dma_start(negm[:], negm_d[:])
        b1B8 = const.tile([128, 2, D], bf16, tag="b1B8")
        nc.sync.dma_start(b1B8[:], b1B_d[:])
        b2col = const.tile([128, 2], f32, tag="b2col")
        nc.sync.dma_start(b2col[:], b2c_d[:])
        mw = const.tile([128, bpc * WIN], bf16, tag="mw")
        nc.sync.dma_start(mw[:], mw_d[:])
        dma_adj(*ADJ_CH[2])
        dma_axtb(0)
        dma_axt(*AXT_CH[2])
        dma_adj(*ADJ_CH[3])
        dma_axtb(4)
        dma_xnb(0)
        dma_adj(*ADJ_CH[4])
        dma_axtb(8)
        dma_xnb(4)
        dma_axtb(12)
        dma_xnb(8)
        dma_xnb(12)

        onescol = const.tile([128, 1], f32, tag="onescol")
        nc.vector.memset(onescol[:], 1.0)
        onesrow = const.tile([1, 128], bf16, tag="onesrow")
        nc.vector.memset(onesrow[:], 1.0)
        sumP = pstage.tile([128, bpc], f32, tag="sumP")
        HB = bpc // 2
        outRa = pstage.tile([1, HB * D], f32, tag="outRa")
        outRb = pstage.tile([1, HB * D], f32, tag="outRb")
        sume = pstage.tile([1, bpc], f32, tag="sume")

        T = {b: {} for b in range(bpc)}

        def emit_p1(b):
            # h1[s,e] = x[s,:] @ W1 ; lhsT = fp8 xT slice, rhs = 16*W1.
            # PSUM = 16*h1 -> fp8 copy (scalar/vector alternating).
            n = n_slots[b]
            at, ao = axt_t[b]
            axt = at[:, ao:ao + 2 * 128 * n].rearrange(
                "p (c s) -> p c s", c=2)
            h1s8 = pmid.tile([128, 4, D], f8, name="h1s8", tag="h1s8")
            for sc in range(n):
                ps_h = psH.tile([128, D], f32, name="ps_h", tag="ps_h")
                nc.tensor.matmul(ps_h[:], axt[:, :, ts(sc, 128)],
                                 W1s8[:, :, :], perf_mode=DR)
                if sc % 2 == 0:
                    nc.scalar.copy(h1s8[:, sc, :], ps_h[:])
                else:
                    nc.vector.tensor_copy(h1s8[:, sc, :], ps_h[:])
            T[b]["h1s8"] = h1s8

        def emit_p2(b):
            # g1 = b1 + adjTw.T @ h1 ; x2 = fp8(relu(256*g1))
            n = n_slots[b]
            at, ao = adj_t[b]
            adjs = at[:, ao:ao + n * 128 * n].rearrange(
                "p (c s) -> p c s", c=n)
            h1s8 = T[b]["h1s8"]
            x2 = pmid.tile([128, 4, D], f8, name="x2", tag="x2")
            for half in range((n + 1) // 2):
                w_ = min(2, n - 2 * half)
                ps_g = psG.tile([128, 2, D], f32, name="ps_g", tag="ps_g")
                if USE_BIAS_MM:
                    # bias first: PSUM = 65536*b1 via a K=1 bf16 matmul
                    nc.tensor.matmul(ps_g[:, 0:w_, :], onesrow[:],
                                     b1r16[:, 0:w_, :], start=True, stop=False)
                for sci in range(w_):
                    sc = 2 * half + sci
                    last = (sci == w_ - 1)
                    # DoubleRow over t-chunk pairs (fp8: 2 k-tiles/inst)
                    for tp in range(n // 2):
                        nc.tensor.matmul(
                            ps_g[:, sci, :],
                            adjs[:, 2 * tp:2 * tp + 2, ts(sc, 128)],
                            h1s8[:, 2 * tp:2 * tp + 2, :],
                            perf_mode=DR,
                            start=(not USE_BIAS_MM and tp == 0),
                            stop=(last and n % 2 == 0 and tp == n // 2 - 1))
                    if n % 2:
                        nc.tensor.matmul(
                            ps_g[:, sci, :],
                            adjs[:, n - 1, ts(sc, 128)],
                            h1s8[:, n - 1, :],
                            start=False,
                            stop=last)
                if USE_BIAS_MM:
                    # x2 = fp8(relu(2^-8 * PSUM)) straight out of PSUM
                    if half == 0:
                        nc.scalar.activation(
                            x2[:, 0:w_, :], ps_g[:, 0:w_, :], AF.Relu,
                            scale=1.0 / 256.0)
                    else:
                        nc.vector.tensor_scalar(
                            x2[:, 2:2 + w_, :], ps_g[:, 0:w_, :],
                            1.0 / 256.0, 0.0, op0=OP.mult, op1=OP.max)
                else:
                    gt = pmid.tile([128, 2, D], bf16, name="gt", tag="gt")
                    nc.vector.scalar_tensor_tensor(
                        gt[:, 0:w_, :], ps_g[:, 0:w_, :], 1.0 / 256.0,
                        b1B8[:, 0:w_, :], op0=OP.mult, op1=OP.add)
                    if half == 0:
                        nc.scalar.activation(
                            x2[:, 0:w_, :], gt[:, 0:w_, :], AF.Relu)
                    else:
                        nc.vector.tensor_scalar(
                            x2[:, 2:2 + w_, :], gt[:, 0:w_, :],
                            1.0, 0.0, op0=OP.mult, op1=OP.max)
            T[b]["x2"] = x2

        def emit_p3a(b):
            # window layer: ps_y = (256 x2).T @ (4096 awm) = 2^20 yT
            n = n_slots[b]
            x2 = T[b]["x2"]
            awms = awm[:, awm_off[b]:awm_off[b + 1]].rearrange(
                "p (c w) -> p c w", c=n)
            ps_y = psS.tile([128, 2, WIN], f32, name="ps_y", tag="ps_s")
            for dc in range(2):
                for sp in range(n // 2):
                    nc.tensor.matmul(ps_y[:, dc, :],
                                     x2[:, 2 * sp:2 * sp + 2, ts(dc, 128)],
                                     awms[:, 2 * sp:2 * sp + 2, :],
                                     perf_mode=DR,
                                     start=(sp == 0),
                                     stop=(n % 2 == 0 and sp == n // 2 - 1))
                if n % 2:
                    nc.tensor.matmul(ps_y[:, dc, :],
                                     x2[:, n - 1, ts(dc, 128)],
                                     awms[:, n - 1, :],
                                     start=False, stop=True)
            yTb = psmall.tile([128, 2, WIN], bf16, name="yTb", tag="yTb")
            nc.vector.tensor_scalar(yTb[:], ps_y[:], 2.0 ** -20, 0.0,
                                    op0=OP.mult, op1=OP.add)
            T[b]["yTb"] = yTb

        def emit_p3b(b):
            # ps_z = W2b.T @ yTb = z ; r1 = relu(z + b2) ;
            # xs = sum_w r1*mw -> xsb bf16 [128, 2]
            yTb = T[b]["yTb"]
            ps_z = psS.tile([128, 2, WIN], f32, name="ps_z", tag="ps_s")
            for ec in range(2):
                for dc in range(2):
                    nc.tensor.matmul(ps_z[:, ec, :],
                                     W2b[:, dc, ts(ec, 128)],
                                     yTb[:, dc, :],
                                     start=(dc == 0),
                                     stop=(not USE_FUSED_P3B and dc == 1))
                if USE_FUSED_P3B:
                    # + b2[e] (rank-1: b2 slice as weights x ones) and
                    # -1000 on masked window cols: relu also applies the mask
                    nc.tensor.matmul(ps_z[:, ec, :], b2r[:, ec, :],
                                     onesrow[0:1, 0:WIN],
                                     start=False, stop=False)
                    nc.tensor.matmul(ps_z[:, ec, :], onesrow[:],
                                     negm[:, b * WIN:(b + 1) * WIN],
                                     start=False, stop=True)
            xsb = psmall.tile([128, 2], bf16, name="xsb", tag="xsb")
            if USE_FUSED_P3B:
                for ec in range(2):
                    x3 = psmall.tile([128, WIN], f32, name="x3", tag="x3")
                    xs_f = psmall.tile([128, 1], f32, name="xs_f", tag="xs_f")
                    # out = relu(ps_z) via op0=max; op1 is the REDUCTION op
                    # when accum_out is set: accum = sum(out)
                    nc.vector.tensor_scalar(
                        x3[:], ps_z[:, ec, :], 0.0, None,
                        op0=OP.max, op1=OP.add, accum_out=xs_f[:])
                    nc.gpsimd.tensor_copy(xsb[:, ec:ec + 1], xs_f[:])
            else:
                r1 = psmall.tile([128, 2, WIN], f32, name="r1", tag="r1")
                for ec in range(2):
                    nc.scalar.activation(r1[:, ec, :], ps_z[:, ec, :],
                                         AF.Relu, bias=b2col[:, ec:ec + 1])
                for ec in range(2):
                    x3 = psmall.tile([128, WIN], f32, name="x3", tag="x3")
                    xs_f = psmall.tile([128, 1], f32, name="xs_f", tag="xs_f")
                    nc.vector.scalar_tensor_tensor(
                        x3[:], r1[:, ec, :], 1.0,
                        mw[:, b * WIN:(b + 1) * WIN],
                        op0=OP.mult, op1=OP.mult, accum_out=xs_f[:])
                    nc.gpsimd.tensor_copy(xsb[:, ec:ec + 1], xs_f[:])
            T[b]["xsb"] = xsb

        def emit_p4l(b):
            # transposed logits: ps_lT[:, tc] = sum_dc axtb[:,dc,tc*128:].T @ xs
            at, ao = axtb_t[b]
            axtbs = at[:, ao:ao + 2 * S].rearrange("p (c s) -> p c s", c=2)
            xsb = T[b]["xsb"]
            lo = psLO.tile([128, 260], f32, name="ps_lo", tag="ps_lo")
            for tc_ in range(4):
                for dc in range(2):
                    nc.tensor.matmul(lo[:, tc_:tc_ + 1],
                                     axtbs[:, dc, ts(tc_, 128)],
                                     xsb[:, dc:dc + 1],
                                     start=(dc == 0), stop=(dc == 1))
            pexp = psmall.tile([128, 4], bf16, name="pexp", tag="pexp")
            nc.scalar.activation(pexp[:], lo[:, 0:4], AF.Exp,
                                 accum_out=sumP[:, b:b + 1])
            T[b]["pexp"] = pexp
            T[b]["lo"] = lo

        def emit_ws(b):
            # weighted sum: out[1, D] = sum_tc pexp[:, tc].T @ xnb[:, tc, :]
            at, ao = xnb_t[b]
            xns = at[:, ao:ao + 2 * S].rearrange("p (c s) -> p c s", c=4)
            pexp = T[b]["pexp"]
            lo = T[b]["lo"]
            for tc_ in range(4):
                nc.tensor.matmul(lo[0:1, 4:4 + D], pexp[:, tc_:tc_ + 1],
                                 xns[:, tc_, :],
                                 start=(tc_ == 0), stop=(tc_ == 3))
            outR = outRa if b < HB else outRb
            bo = b if b < HB else b - HB
            if b % 2 == 0:
                nc.scalar.copy(outR[:, bo * D:(bo + 1) * D], lo[0:1, 4:4 + D])
            else:
                nc.vector.tensor_copy(outR[:, bo * D:(bo + 1) * D],
                                      lo[0:1, 4:4 + D])

        for i in range(NSTEP):
            if 0 <= i - LAG_P4 < bpc:
                emit_p4l(i - LAG_P4)
            if i < bpc:
                emit_p1(i)
            if 0 <= i - LAG_P3 < bpc:
                emit_p3a(i - LAG_P3)
            if 0 <= i - LAG_P2 < bpc:
                emit_p2(i - LAG_P2)
            if 0 <= i - LAG_P3 < bpc:
                emit_p3b(i - LAG_P3)
            if 0 <= i - LAG_WS < bpc:
                emit_ws(i - LAG_WS)
            if i - LAG_WS == HB - 1:
                # first-half output rows are final: overlap their store
                nc.sync.dma_start(outR_d[:, 0:HB * D], outRa[:])

        # sume[1, bpc] = ones.T @ sumP  (single f32 matmul)
        ps_s = psLO.tile([128, 260], f32, name="ps_sm", tag="ps_lo")
        nc.tensor.matmul(ps_s[0:1, 0:bpc], onescol[:], sumP[:])
        nc.scalar.copy(sume[:], ps_s[0:1, 0:bpc])

        nc.sync.dma_start(sume_d[:], sume[:])
        nc.sync.dma_start(outR_d[:, HB * D:], outRb[:])

    nc.compile()
    return nc


def _f8(x):
    return np.clip(x, -240.0, 240.0).astype(F8)


def _plan(inputs):
    """Host-side preprocessing: fold position weight / degree norm / fp8
    scales; sort samples by chunk count into per-core slots; pack per-core
    DRAM blobs. order[b*NCORES + c] is the original sample index placed in
    slot b of core c."""
    text_out = np.asarray(inputs["text_out"], dtype=np.float32)
    adj = np.asarray(inputs["adj"], dtype=np.float32)
    W1 = np.asarray(inputs["W1"], dtype=np.float32)
    b1 = np.asarray(inputs["b1"], dtype=np.float32)
    W2 = np.asarray(inputs["W2"], dtype=np.float32)
    b2 = np.asarray(inputs["b2"], dtype=np.float32)
    tl = np.asarray(inputs["text_len"]).astype(np.int64)
    al = np.asarray(inputs["aspect_len"]).astype(np.int64)
    ll = np.asarray(inputs["left_len"]).astype(np.int64)

    n_all = np.minimum(4, np.maximum(2, (tl + 127) // 128)).astype(np.int64)
    # descending: big-n slots first (denser warmup, lighter pipeline drain)
    order = np.argsort(-n_all, kind="stable")       # [B]
    n_slots = tuple(int(n_all[order[b * NCORES:(b + 1) * NCORES]].max())
                    for b in range(BPC))
    axt_off, adj_off, awm_off = _offsets(n_slots)

    j = np.arange(S)[None, :]
    start = ll[:, None]
    end = (ll + al - 1)[:, None]
    ctxlen = (tl - al).astype(np.float32)[:, None]
    w = np.where(j < start, 1.0 - (start - j) / ctxlen,
                 np.where(j <= end, 0.0,
                          np.where(j < tl[:, None], 1.0 - (j - end) / ctxlen,
                                   0.0))).astype(np.float32)      # [B,S]
    dinv = (1.0 / (adj.sum(axis=2) + 1.0)).astype(np.float32)     # [B,S]

    # transposed adjacency, position weight (t) and 1/den (s) folded, *4096
    adjTw = adj.transpose(0, 2, 1) * (4096.0 * w[:, :, None]) * dinv[:, None, :]
    adj8f = _f8(adjTw)                  # [B, t(S), s(S)]
    xT = text_out.transpose(0, 2, 1)    # [B, D, S]
    axt8f = _f8(xT)
    axtbf = xT.astype(BF)
    xnbf = text_out.astype(BF)          # [B, S(t), D]

    win = np.clip(ll[:, None] + np.arange(WIN)[None, :], 0, S - 1)  # [B,WIN]
    adj_win = np.take_along_axis(adj, win[:, :, None], axis=1)      # [B,WIN,S]
    dinvW = np.take_along_axis(dinv, win, axis=1)                   # [B,WIN]
    adjWTw = (adj_win.transpose(0, 2, 1) * (4096.0 * w[:, :, None])
              * dinvW[:, None, :])
    awm8f = _f8(adjWTw)                 # [B, s(S), WIN]

    # 0 on active window cols, -1000 on masked: relu applies the mask
    negmf = np.where(np.arange(WIN)[None, :] < al[:, None],
                     0.0, -1000.0).astype(BF)       # [B, WIN]

    W1s8 = _f8(np.ascontiguousarray(
        (16.0 * W1).reshape(2, 128, D).transpose(1, 0, 2)))
    W2bb = np.ascontiguousarray(
        W2.reshape(2, 128, D).transpose(1, 0, 2)).astype(BF)
    b1r16 = np.ascontiguousarray(
        (65536.0 * np.tile(b1, 2)).reshape(1, 2, D)).astype(BF)
    b2rr = np.ascontiguousarray(b2.reshape(1, 2, 128)).astype(BF)
    b1B8 = np.ascontiguousarray(np.broadcast_to(
        (256.0 * np.tile(b1, 2)).reshape(1, 2, D), (128, 2, D))).astype(BF)
    b2col = np.ascontiguousarray(b2.reshape(2, 128).T).astype(np.float32)

    in_maps = []
    for c in range(NCORES):
        idx = order[np.arange(BPC) * NCORES + c]   # slot b -> order[b*8+c]
        axt_p = np.empty((128, axt_off[-1]), dtype=F8)
        adj_p = np.empty((128, adj_off[-1]), dtype=F8)
        awm_p = np.empty((128, awm_off[-1]), dtype=F8)
        negm_p = np.empty((1, BPC * WIN), dtype=BF)
        mw_p = np.empty((128, BPC * WIN), dtype=BF)
        axtb_p = np.empty((128, BPC * 2 * S), dtype=BF)
        xnb_p = np.empty((128, BPC * 2 * S), dtype=BF)
        for b in range(BPC):
            bi = idx[b]
            n = n_slots[b]
            sa = 128 * n
            # axt8: [D, S] -> [128, 2(dc), 128n] -> flat
            axt_p[:, axt_off[b]:axt_off[b + 1]] = (
                axt8f[bi].reshape(2, 128, S)[:, :, :sa]
                .transpose(1, 0, 2).reshape(128, 2 * sa))
            # adj8: [t(S), s(S)] -> [128(t in chunk), n(tc), 128n(s)]
            adj_p[:, adj_off[b]:adj_off[b + 1]] = (
                adj8f[bi].reshape(4, 128, S)[:n, :, :sa]
                .transpose(1, 0, 2).reshape(128, n * sa))
            # awm: [s(S), WIN] -> [128, n(sc), WIN]
            awm_p[:, awm_off[b]:awm_off[b + 1]] = (
                awm8f[bi].reshape(4, 128, WIN)[:n]
                .transpose(1, 0, 2).reshape(128, n * WIN))
            negm_p[0, b * WIN:(b + 1) * WIN] = negmf[bi]
            mw_p[:, b * WIN:(b + 1) * WIN] = (negmf[bi] == 0).astype(BF)[None, :]
            # axtb: [D, S] -> [128, 2(dc), S]
            axtb_p[:, b * 2 * S:(b + 1) * 2 * S] = (
                axtbf[bi].reshape(2, 128, S)
                .transpose(1, 0, 2).reshape(128, 2 * S))
            # xnb: [S(t), D] -> [128(t in chunk), 4(tc), D]
            xnb_p[:, b * 2 * S:(b + 1) * 2 * S] = (
                xnbf[bi].reshape(4, 128, D)
                .transpose(1, 0, 2).reshape(128, 4 * D))
        in_maps.append({
            "axt8": np.ascontiguousarray(axt_p),
            "adj8": np.ascontiguousarray(adj_p),
            "awm": np.ascontiguousarray(awm_p),
            "negm": np.ascontiguousarray(negm_p),
            "axtb": np.ascontiguousarray(axtb_p),
            "xnb": np.ascontiguousarray(xnb_p),
            "W1s8": W1s8, "W2b": W2bb, "b1r16": b1r16, "b2r": b2rr,
            "b1B8": b1B8, "b2col": b2col, "mw": mw_p,
        })
    return in_maps, n_slots, order


def _assemble(results, order):
    out = np.empty((B, D), dtype=np.float32)
    for c in range(NCORES):
        outR = results[c]["outR"].reshape(BPC, D)
        sume = results[c]["sume"].reshape(-1)  # [BPC]
        for b in range(BPC):
            out[order[b * NCORES + c]] = outR[b] / sume[b]
    return out


def kernel(**inputs):
    from concourse.bass_utils import run_bass_kernel_spmd

    in_maps, n_slots, order = _plan(inputs)
    key = (BPC, n_slots)
    if key not in _nc_cache:
        _nc_cache[key] = _build_nc(BPC, n_slots)
    nc = _nc_cache[key]
    res = run_bass_kernel_spmd(nc, in_maps, list(range(NCORES)))
    return _assemble(res.results, order)


# revision 33
# speedup vs baseline: 1.1887x; 1.1887x over previous
"""ASGCN unit kernel for 8 Trainium2 NeuronCores (data-parallel over batch).

Contract: kernel(**inputs) takes the FULL unsharded inputs and returns the
FULL [128, 256] float32 output. Batch is sharded 16 samples/core across 8
cores; all parameters are replicated.

v2 design (evolved from the fp8 baseline after trace analysis):
  - position_weight, degree norm and fp8 scales are folded on host into the
    transposed adjacency (adjTw = adjT * 4096 * w[t] * dinv[s]) exactly as
    before; samples are sorted by n = ceil(text_len/128) into per-core slots
    sharing one slot->n pattern (SPMD) so matmuls skip structurally-zero
    128-chunks.
  - ALL inputs are shipped as a few large per-core packed DRAM blobs and
    loaded with ~18 big DMAs on the sync queue (the old per-sample DMA
    scheme kept the sync engine ~90% busy generating descriptors).
  - attention is restructured to be (almost) all-tensor:
      * logits computed TRANSPOSED: ps_lT[t,1] per 128-chunk via 8 tiny
        matmuls (lhsT = bf16 xT chunk, rhs = xs column),
      * exp on [128,4] (scalar engine, 128-partition utilization) with the
        per-partition accumulator collected into sumP[:, b],
      * weighted sum as 4 N=256 bf16 matmuls against a NORMAL-layout bf16
        copy of text_out (new input), giving the output row [1,256] in PSUM,
      * sum(exp) for all 16 samples reduced with ONE final f32 matmul
        (ones.T @ sumP).
    This removes the p-broadcast matmul, the [128,512] scalar copy and the
    two [128,512] vector accumulate-STTs per sample of the old design.
  - GCN layer 2 (window) runs fp8 for the adjacency contraction then bf16
    for the W2 matmul (better accuracy than the old all-fp8 path).
  - PSUM->SBUF epilogues are spread across scalar/vector; gpsimd (no PSUM
    port) takes the SBUF-only relu+cast work.
"""

import sys

if "/opt/trn_rl_repo" not in sys.path:
    sys.path.insert(0, "/opt/trn_rl_repo")

import numpy as np
import ml_dtypes

B, S, D, WIN = 128, 512, 256, 8
NCORES = 8
BPC = B // NCORES  # samples per core
BF = ml_dtypes.bfloat16
F8 = ml_dtypes.float8_e4m3  # TRN fp8e4: max +-240

_nc_cache = {}
USE_BIAS_MM = False
USE_FUSED_P3B = True


def _offsets(n_slots):
    """Per-slot element offsets (per partition) into the packed blobs."""
    axt_off, adj_off, awm_off = [0], [0], [0]
    for n in n_slots:
        axt_off.append(axt_off[-1] + 2 * 128 * n)      # [128, 2, 128n] fp8
        adj_off.append(adj_off[-1] + n * 128 * n)      # [128, n, 128n] fp8
        awm_off.append(awm_off[-1] + n * WIN)          # [128, n, WIN] fp8
    return axt_off, adj_off, awm_off


def _build_nc(bpc, n_slots):
    from contextlib import ExitStack

    import concourse.bass as bass
    import concourse.tile as tile
    from concourse import bacc, mybir

    dt = mybir.dt
    f32, bf16, f8 = dt.float32, dt.bfloat16, dt.float8e4
    AF = mybir.ActivationFunctionType
    OP = mybir.AluOpType
    DR = mybir.MatmulPerfMode.DoubleRow
    ts = bass.ts

    axt_off, adj_off, awm_off = _offsets(n_slots)

    nc = bacc.Bacc("TRN2", target_bir_lowering=False, debug=False,
                   num_devices=NCORES)

    # --- DRAM parameters: packed per-core blobs ---
    axt8_d = nc.declare_dram_parameter("axt8", [128, axt_off[-1]], f8,
                                       isOutput=False)
    adj8_d = nc.declare_dram_parameter("adj8", [128, adj_off[-1]], f8,
                                       isOutput=False)
    awm_d = nc.declare_dram_parameter("awm", [128, awm_off[-1]], f8,
                                      isOutput=False)
    negm_d = nc.declare_dram_parameter("negm", [1, bpc * WIN], bf16,
                                       isOutput=False)
    axtb_d = nc.declare_dram_parameter("axtb", [128, bpc * 2 * S], bf16,
                                       isOutput=False)
    xnb_d = nc.declare_dram_parameter("xnb", [128, bpc * 2 * S], bf16,
                                      isOutput=False)
    W1_d = nc.declare_dram_parameter("W1s8", [128, 2, D], f8, isOutput=False)
    W2_d = nc.declare_dram_parameter("W2b", [128, 2, D], bf16, isOutput=False)
    b1_d = nc.declare_dram_parameter("b1r16", [1, 2, D], bf16, isOutput=False)
    b2_d = nc.declare_dram_parameter("b2r", [1, 2, 128], bf16, isOutput=False)
    b1B_d = nc.declare_dram_parameter("b1B8", [128, 2, D], bf16, isOutput=False)
    b2c_d = nc.declare_dram_parameter("b2col", [128, 2], f32, isOutput=False)
    mw_d = nc.declare_dram_parameter("mw", [128, bpc * WIN], bf16, isOutput=False)
    outR_d = nc.declare_dram_parameter("outR", [1, bpc * D], f32,
                                       isOutput=True)
    sume_d = nc.declare_dram_parameter("sume", [1, bpc], f32, isOutput=True)

    LAG_P2, LAG_P3, LAG_P4, LAG_WS = 1, 2, 3, 3
    NSTEP = bpc + LAG_WS

    with tile.TileContext(nc) as tc, ExitStack() as ctx:
        const = ctx.enter_context(tc.tile_pool(name="const", bufs=1))
        pmid = ctx.enter_context(tc.tile_pool(name="pmid", bufs=6))
        psmall = ctx.enter_context(tc.tile_pool(name="psmall", bufs=8))
        pstage = ctx.enter_context(tc.tile_pool(name="pstage", bufs=1))
        psH = ctx.enter_context(tc.tile_pool(name="psH", bufs=2, space="PSUM"))
        psG = ctx.enter_context(tc.tile_pool(name="psG", bufs=2, space="PSUM"))
        psS = ctx.enter_context(tc.tile_pool(name="psS", bufs=2, space="PSUM"))
        # logits column [:, 0:4] and the ws output row [0:1, 4:260] share one
        # bank-sized tile (they are serially dependent through exp anyway)
        psLO = ctx.enter_context(tc.tile_pool(name="psLO", bufs=2,
                                              space="PSUM"))

        # ---- input SBUF blobs + the DMA schedule (sync queue only) ----
        W1s8 = const.tile([128, 2, D], f8, tag="W1s8")
        nc.sync.dma_start(W1s8[:], W1_d[:])

        # chunked blobs: independent tiles so readers only wait their chunk;
        # leading chunks are small so the pipeline starts ASAP
        AXT_CH = [(0, 2), (2, 6), (8, 8)]
        ADJ_CH = [(0, 2), (2, 2), (4, 4), (8, 4), (12, 4)]
        ABC, XNC = 4, 4  # slots per chunk
        axt_t, adj_t, axtb_t, xnb_t = {}, {}, {}, {}

        def dma_axt(c0, cnt):
            e0, e1 = axt_off[c0], axt_off[min(c0 + cnt, bpc)]
            t = const.tile([128, e1 - e0], f8, name=f"axt{c0}", tag=f"axt{c0}")
            nc.sync.dma_start(t[:], axt8_d[:, e0:e1])
            for b in range(c0, min(c0 + cnt, bpc)):
                axt_t[b] = (t, axt_off[b] - e0)

        def dma_adj(c0, cnt):
            e0, e1 = adj_off[c0], adj_off[min(c0 + cnt, bpc)]
            t = const.tile([128, e1 - e0], f8, name=f"adj{c0}", tag=f"adj{c0}")
            nc.sync.dma_start(t[:], adj8_d[:, e0:e1])
            for b in range(c0, min(c0 + cnt, bpc)):
                adj_t[b] = (t, adj_off[b] - e0)

        def dma_axtb(c0):
            e0, e1 = c0 * 2 * S, min(c0 + ABC, bpc) * 2 * S
            t = const.tile([128, e1 - e0], bf16, name=f"axb{c0}",
                           tag=f"axb{c0}")
            nc.sync.dma_start(t[:], axtb_d[:, e0:e1])
            for b in range(c0, min(c0 + ABC, bpc)):
                axtb_t[b] = (t, (b - c0) * 2 * S)

        def dma_xnb(c0):
            e0, e1 = c0 * 2 * S, min(c0 + XNC, bpc) * 2 * S
            t = const.tile([128, e1 - e0], bf16, name=f"xnb{c0}",
                           tag=f"xnb{c0}")
            nc.sync.dma_start(t[:], xnb_d[:, e0:e1])
            for b in range(c0, min(c0 + XNC, bpc)):
                xnb_t[b] = (t, (b - c0) * 2 * S)

        dma_axt(*AXT_CH[0])
        dma_adj(*ADJ_CH[0])
        b1r16 = const.tile([1, 2, D], bf16, tag="b1r16")
        nc.sync.dma_start(b1r16[:], b1_d[:])
        dma_axt(*AXT_CH[1])
        dma_adj(*ADJ_CH[1])
        W2b = const.tile([128, 2, D], bf16, tag="W2b")
        nc.sync.dma_start(W2b[:], W2_d[:])
        b2r = const.tile([1, 2, 128], bf16, tag="b2r")
        nc.sync.dma_start(b2r[:], b2_d[:])
        awm = const.tile([128, awm_off[-1]], f8, tag="awm")
        nc.sync.dma_start(awm[:], awm_d[:])
        negm = const.tile([1, bpc * WIN], bf16, tag="negm")
        nc.sync.dma_start(negm[:], negm_d[:])
        b1B8 = const.tile([128, 2, D], bf16, tag="b1B8")
        nc.sync.dma_start(b1B8[:], b1B_d[:])
        b2col = const.tile([128, 2], f32, tag="b2col")
        nc.sync.dma_start(b2col[:], b2c_d[:])
        mw = const.tile([128, bpc * WIN], bf16, tag="mw")
        nc.sync.dma_start(mw[:], mw_d[:])
        dma_adj(*ADJ_CH[2])
        dma_axtb(0)
        dma_axt(*AXT_CH[2])
        dma_adj(*ADJ_CH[3])
        dma_axtb(4)
        dma_xnb(0)
        dma_adj(*ADJ_CH[4])
        dma_axtb(8)
        dma_xnb(4)
        dma_axtb(12)
        dma_xnb(8)
        dma_xnb(12)

        onescol = const.tile([128, 1], f32, tag="onescol")
        nc.vector.memset(onescol[:], 1.0)
        onesrow = const.tile([1, 128], bf16, tag="onesrow")
        nc.vector.memset(onesrow[:], 1.0)
        sumP = pstage.tile([128, bpc], f32, tag="sumP")
        HB = bpc // 2
        outRa = pstage.tile([1, HB * D], f32, tag="outRa")
        outRb = pstage.tile([1, HB * D], f32, tag="outRb")
        sume = pstage.tile([1, bpc], f32, tag="sume")

        T = {b: {} for b in range(bpc)}

        def emit_p1(b):
            # h1[s,e] = x[s,:] @ W1 ; lhsT = fp8 xT slice, rhs = 16*W1.
            # PSUM = 16*h1 -> fp8 copy (scalar/vector alternating).
            n = n_slots[b]
            at, ao = axt_t[b]
            axt = at[:, ao:ao + 2 * 128 * n].rearrange(
                "p (c s) -> p c s", c=2)
            h1s8 = pmid.tile([128, 4, D], f8, name="h1s8", tag="h1s8")
            for sc in range(n):
                ps_h = psH.tile([128, D], f32, name="ps_h", tag="ps_h")
                nc.tensor.matmul(ps_h[:], axt[:, :, ts(sc, 128)],
                                 W1s8[:, :, :], perf_mode=DR)
                if sc % 2 == 0:
                    nc.scalar.copy(h1s8[:, sc, :], ps_h[:])
                else:
                    nc.vector.tensor_copy(h1s8[:, sc, :], ps_h[:])
            T[b]["h1s8"] = h1s8

        def emit_p2h(b, half):
            # g1 = b1 + adjTw.T @ h1 ; x2 = fp8(relu(256*g1))
            n = n_slots[b]
            if half >= (n + 1) // 2:
                return
            at, ao = adj_t[b]
            adjs = at[:, ao:ao + n * 128 * n].rearrange(
                "p (c s) -> p c s", c=n)
            h1s8 = T[b]["h1s8"]
            if half == 0:
                x2 = pmid.tile([128, 4, D], f8, name="x2", tag="x2")
                T[b]["x2"] = x2
            else:
                x2 = T[b]["x2"]
            if True:
                w_ = min(2, n - 2 * half)
                ps_g = psG.tile([128, 2, D], f32, name="ps_g", tag="ps_g")
                if USE_BIAS_MM:
                    # bias first: PSUM = 65536*b1 via a K=1 bf16 matmul
                    nc.tensor.matmul(ps_g[:, 0:w_, :], onesrow[:],
                                     b1r16[:, 0:w_, :], start=True, stop=False)
                for sci in range(w_):
                    sc = 2 * half + sci
                    # DoubleRow over t-chunk pairs (fp8: 2 k-tiles/inst);
                    # each sci slice is its own open/close psum group
                    for tp in range(n // 2):
                        nc.tensor.matmul(
                            ps_g[:, sci, :],
                            adjs[:, 2 * tp:2 * tp + 2, ts(sc, 128)],
                            h1s8[:, 2 * tp:2 * tp + 2, :],
                            perf_mode=DR,
                            start=(not USE_BIAS_MM and tp == 0),
                            stop=(n % 2 == 0 and tp == n // 2 - 1))
                    if n % 2:
                        nc.tensor.matmul(
                            ps_g[:, sci, :],
                            adjs[:, n - 1, ts(sc, 128)],
                            h1s8[:, n - 1, :],
                            start=False,
                            stop=True)
                if USE_BIAS_MM:
                    # x2 = fp8(relu(2^-8 * PSUM)) straight out of PSUM
                    if half == 0:
                        nc.scalar.activation(
                            x2[:, 0:w_, :], ps_g[:, 0:w_, :], AF.Relu,
                            scale=1.0 / 256.0)
                    else:
                        nc.vector.tensor_scalar(
                            x2[:, 2:2 + w_, :], ps_g[:, 0:w_, :],
                            1.0 / 256.0, 0.0, op0=OP.mult, op1=OP.max)
                else:
                    gt = pmid.tile([128, 2, D], bf16, name="gt", tag="gt")
                    nc.vector.scalar_tensor_tensor(
                        gt[:, 0:w_, :], ps_g[:, 0:w_, :], 1.0 / 256.0,
                        b1B8[:, 0:w_, :], op0=OP.mult, op1=OP.add)
                    if half == 0:
                        nc.scalar.activation(
                            x2[:, 0:w_, :], gt[:, 0:w_, :], AF.Relu)
                    else:
                        nc.vector.tensor_scalar(
                            x2[:, 2:2 + w_, :], gt[:, 0:w_, :],
                            1.0, 0.0, op0=OP.mult, op1=OP.max)

        def emit_p3a(b):
            # window layer: ps_y = (256 x2).T @ (4096 awm) = 2^20 yT
            n = n_slots[b]
            x2 = T[b]["x2"]
            awms = awm[:, awm_off[b]:awm_off[b + 1]].rearrange(
                "p (c w) -> p c w", c=n)
            ps_y = psS.tile([128, 2, WIN], f32, name="ps_y", tag="ps_s")
            for dc in range(2):
                for sp in range(n // 2):
                    nc.tensor.matmul(ps_y[:, dc, :],
                                     x2[:, 2 * sp:2 * sp + 2, ts(dc, 128)],
                                     awms[:, 2 * sp:2 * sp + 2, :],
                                     perf_mode=DR,
                                     start=(sp == 0),
                                     stop=(n % 2 == 0 and sp == n // 2 - 1))
                if n % 2:
                    nc.tensor.matmul(ps_y[:, dc, :],
                                     x2[:, n - 1, ts(dc, 128)],
                                     awms[:, n - 1, :],
                                     start=False, stop=True)
            yTb = psmall.tile([128, 2, WIN], bf16, name="yTb", tag="yTb")
            nc.vector.tensor_scalar(yTb[:], ps_y[:], 2.0 ** -20, 0.0,
                                    op0=OP.mult, op1=OP.add)
            T[b]["yTb"] = yTb

        def emit_p3b(b):
            # ps_z = W2b.T @ yTb = z ; r1 = relu(z + b2) ;
            # xs = sum_w r1*mw -> xsb bf16 [128, 2]
            yTb = T[b]["yTb"]
            ps_z = psS.tile([128, 2, WIN], f32, name="ps_z", tag="ps_s")
            for ec in range(2):
                for dc in range(2):
                    nc.tensor.matmul(ps_z[:, ec, :],
                                     W2b[:, dc, ts(ec, 128)],
                                     yTb[:, dc, :],
                                     start=(dc == 0),
                                     stop=(not USE_FUSED_P3B and dc == 1))
                if USE_FUSED_P3B:
                    # + b2[e] (rank-1: b2 slice as weights x ones) and
                    # -1000 on masked window cols: relu also applies the mask
                    nc.tensor.matmul(ps_z[:, ec, :], b2r[:, ec, :],
                                     onesrow[0:1, 0:WIN],
                                     start=False, stop=False)
                    nc.tensor.matmul(ps_z[:, ec, :], onesrow[:],
                                     negm[:, b * WIN:(b + 1) * WIN],
                                     start=False, stop=True)
            xsb = psmall.tile([128, 2], bf16, name="xsb", tag="xsb")
            if USE_FUSED_P3B:
                for ec in range(2):
                    x3 = psmall.tile([128, WIN], f32, name="x3", tag="x3")
                    xs_f = psmall.tile([128, 1], f32, name="xs_f", tag="xs_f")
                    # out = relu(ps_z) via op0=max; op1 is the REDUCTION op
                    # when accum_out is set: accum = sum(out)
                    nc.vector.tensor_scalar(
                        x3[:], ps_z[:, ec, :], 0.0, None,
                        op0=OP.max, op1=OP.add, accum_out=xs_f[:])
                    nc.gpsimd.tensor_copy(xsb[:, ec:ec + 1], xs_f[:])
            else:
                r1 = psmall.tile([128, 2, WIN], f32, name="r1", tag="r1")
                for ec in range(2):
                    nc.scalar.activation(r1[:, ec, :], ps_z[:, ec, :],
                                         AF.Relu, bias=b2col[:, ec:ec + 1])
                for ec in range(2):
                    x3 = psmall.tile([128, WIN], f32, name="x3", tag="x3")
                    xs_f = psmall.tile([128, 1], f32, name="xs_f", tag="xs_f")
                    nc.vector.scalar_tensor_tensor(
                        x3[:], r1[:, ec, :], 1.0,
                        mw[:, b * WIN:(b + 1) * WIN],
                        op0=OP.mult, op1=OP.mult, accum_out=xs_f[:])
                    nc.gpsimd.tensor_copy(xsb[:, ec:ec + 1], xs_f[:])
            T[b]["xsb"] = xsb

        def emit_p4l(b):
            # transposed logits: ps_lT[:, tc] = sum_dc axtb[:,dc,tc*128:].T @ xs
            at, ao = axtb_t[b]
            axtbs = at[:, ao:ao + 2 * S].rearrange("p (c s) -> p c s", c=2)
            xsb = T[b]["xsb"]
            lo = psLO.tile([128, 260], f32, name="ps_lo", tag="ps_lo")
            for tc_ in range(4):
                for dc in range(2):
                    nc.tensor.matmul(lo[:, tc_:tc_ + 1],
                                     axtbs[:, dc, ts(tc_, 128)],
                                     xsb[:, dc:dc + 1],
                                     start=(dc == 0), stop=(dc == 1))
            pexp = psmall.tile([128, 4], bf16, name="pexp", tag="pexp")
            nc.scalar.activation(pexp[:], lo[:, 0:4], AF.Exp,
                                 accum_out=sumP[:, b:b + 1])
            T[b]["pexp"] = pexp
            T[b]["lo"] = lo

        def emit_ws(b):
            # weighted sum: out[1, D] = sum_tc pexp[:, tc].T @ xnb[:, tc, :]
            at, ao = xnb_t[b]
            xns = at[:, ao:ao + 2 * S].rearrange("p (c s) -> p c s", c=4)
            pexp = T[b]["pexp"]
            lo = T[b]["lo"]
            for tc_ in range(4):
                nc.tensor.matmul(lo[0:1, 4:4 + D], pexp[:, tc_:tc_ + 1],
                                 xns[:, tc_, :],
                                 start=(tc_ == 0), stop=(tc_ == 3))
            outR = outRa if b < HB else outRb
            bo = b if b < HB else b - HB
            if b % 2 == 0:
                nc.scalar.copy(outR[:, bo * D:(bo + 1) * D], lo[0:1, 4:4 + D])
            else:
                nc.vector.tensor_copy(outR[:, bo * D:(bo + 1) * D],
                                      lo[0:1, 4:4 + D])

        for i in range(NSTEP):
            if 0 <= i - LAG_P4 < bpc:
                emit_p4l(i - LAG_P4)
            if i < bpc:
                emit_p1(i)
            if 0 <= i - LAG_P3 < bpc:
                emit_p3a(i - LAG_P3)
            if 0 <= i - LAG_P2 < bpc:
                emit_p2h(i - LAG_P2, 0)
            if 0 <= i - LAG_P3 < bpc:
                emit_p3b(i - LAG_P3)
            if 0 <= i - LAG_P2 < bpc:
                emit_p2h(i - LAG_P2, 1)
            if 0 <= i - LAG_WS < bpc:
                emit_ws(i - LAG_WS)
            if i - LAG_WS == HB - 1:
                # first-half output rows are final: overlap their store
                nc.sync.dma_start(outR_d[:, 0:HB * D], outRa[:])

        # sume[1, bpc] = ones.T @ sumP  (single f32 matmul)
        ps_s = psLO.tile([128, 260], f32, name="ps_sm", tag="ps_lo")
        nc.tensor.matmul(ps_s[0:1, 0:bpc], onescol[:], sumP[:])
        nc.scalar.copy(sume[:], ps_s[0:1, 0:bpc])

        nc.sync.dma_start(sume_d[:], sume[:])
        nc.sync.dma_start(outR_d[:, HB * D:], outRb[:])

    nc.compile()
    return nc


def _f8(x):
    return np.clip(x, -240.0, 240.0).astype(F8)


def _plan(inputs):
    """Host-side preprocessing: fold position weight / degree norm / fp8
    scales; sort samples by chunk count into per-core slots; pack per-core
    DRAM blobs. order[b*NCORES + c] is the original sample index placed in
    slot b of core c."""
    text_out = np.asarray(inputs["text_out"], dtype=np.float32)
    adj = np.asarray(inputs["adj"], dtype=np.float32)
    W1 = np.asarray(inputs["W1"], dtype=np.float32)
    b1 = np.asarray(inputs["b1"], dtype=np.float32)
    W2 = np.asarray(inputs["W2"], dtype=np.float32)
    b2 = np.asarray(inputs["b2"], dtype=np.float32)
    tl = np.asarray(inputs["text_len"]).astype(np.int64)
    al = np.asarray(inputs["aspect_len"]).astype(np.int64)
    ll = np.asarray(inputs["left_len"]).astype(np.int64)

    n_all = np.minimum(4, np.maximum(2, (tl + 127) // 128)).astype(np.int64)
    # descending: big-n slots first (denser warmup, lighter pipeline drain)
    order = np.argsort(-n_all, kind="stable")       # [B]
    n_slots = tuple(int(n_all[order[b * NCORES:(b + 1) * NCORES]].max())
                    for b in range(BPC))
    axt_off, adj_off, awm_off = _offsets(n_slots)

    j = np.arange(S)[None, :]
    start = ll[:, None]
    end = (ll + al - 1)[:, None]
    ctxlen = (tl - al).astype(np.float32)[:, None]
    w = np.where(j < start, 1.0 - (start - j) / ctxlen,
                 np.where(j <= end, 0.0,
                          np.where(j < tl[:, None], 1.0 - (j - end) / ctxlen,
                                   0.0))).astype(np.float32)      # [B,S]
    dinv = (1.0 / (adj.sum(axis=2) + 1.0)).astype(np.float32)     # [B,S]

    # transposed adjacency, position weight (t) and 1/den (s) folded, *4096
    adjTw = adj.transpose(0, 2, 1) * (4096.0 * w[:, :, None]) * dinv[:, None, :]
    adj8f = _f8(adjTw)                  # [B, t(S), s(S)]
    xT = text_out.transpose(0, 2, 1)    # [B, D, S]
    axt8f = _f8(xT)
    axtbf = xT.astype(BF)
    xnbf = text_out.astype(BF)          # [B, S(t), D]

    win = np.clip(ll[:, None] + np.arange(WIN)[None, :], 0, S - 1)  # [B,WIN]
    adj_win = np.take_along_axis(adj, win[:, :, None], axis=1)      # [B,WIN,S]
    dinvW = np.take_along_axis(dinv, win, axis=1)                   # [B,WIN]
    adjWTw = (adj_win.transpose(0, 2, 1) * (4096.0 * w[:, :, None])
              * dinvW[:, None, :])
    awm8f = _f8(adjWTw)                 # [B, s(S), WIN]

    # 0 on active window cols, -1000 on masked: relu applies the mask
    negmf = np.where(np.arange(WIN)[None, :] < al[:, None],
                     0.0, -1000.0).astype(BF)       # [B, WIN]

    W1s8 = _f8(np.ascontiguousarray(
        (16.0 * W1).reshape(2, 128, D).transpose(1, 0, 2)))
    W2bb = np.ascontiguousarray(
        W2.reshape(2, 128, D).transpose(1, 0, 2)).astype(BF)
    b1r16 = np.ascontiguousarray(
        (65536.0 * np.tile(b1, 2)).reshape(1, 2, D)).astype(BF)
    b2rr = np.ascontiguousarray(b2.reshape(1, 2, 128)).astype(BF)
    b1B8 = np.ascontiguousarray(np.broadcast_to(
        (256.0 * np.tile(b1, 2)).reshape(1, 2, D), (128, 2, D))).astype(BF)
    b2col = np.ascontiguousarray(b2.reshape(2, 128).T).astype(np.float32)

    in_maps = []
    for c in range(NCORES):
        idx = order[np.arange(BPC) * NCORES + c]   # slot b -> order[b*8+c]
        axt_p = np.empty((128, axt_off[-1]), dtype=F8)
        adj_p = np.empty((128, adj_off[-1]), dtype=F8)
        awm_p = np.empty((128, awm_off[-1]), dtype=F8)
        negm_p = np.empty((1, BPC * WIN), dtype=BF)
        mw_p = np.empty((128, BPC * WIN), dtype=BF)
        axtb_p = np.empty((128, BPC * 2 * S), dtype=BF)
        xnb_p = np.empty((128, BPC * 2 * S), dtype=BF)
        for b in range(BPC):
            bi = idx[b]
            n = n_slots[b]
            sa = 128 * n
            # axt8: [D, S] -> [128, 2(dc), 128n] -> flat
            axt_p[:, axt_off[b]:axt_off[b + 1]] = (
                axt8f[bi].reshape(2, 128, S)[:, :, :sa]
                .transpose(1, 0, 2).reshape(128, 2 * sa))
            # adj8: [t(S), s(S)] -> [128(t in chunk), n(tc), 128n(s)]
            adj_p[:, adj_off[b]:adj_off[b + 1]] = (
                adj8f[bi].reshape(4, 128, S)[:n, :, :sa]
                .transpose(1, 0, 2).reshape(128, n * sa))
            # awm: [s(S), WIN] -> [128, n(sc), WIN]
            awm_p[:, awm_off[b]:awm_off[b + 1]] = (
                awm8f[bi].reshape(4, 128, WIN)[:n]
                .transpose(1, 0, 2).reshape(128, n * WIN))
            negm_p[0, b * WIN:(b + 1) * WIN] = negmf[bi]
            mw_p[:, b * WIN:(b + 1) * WIN] = (negmf[bi] == 0).astype(BF)[None, :]
            # axtb: [D, S] -> [128, 2(dc), S]
            axtb_p[:, b * 2 * S:(b + 1) * 2 * S] = (
                axtbf[bi].reshape(2, 128, S)
                .transpose(1, 0, 2).reshape(128, 2 * S))
            # xnb: [S(t), D] -> [128(t in chunk), 4(tc), D]
            xnb_p[:, b * 2 * S:(b + 1) * 2 * S] = (
                xnbf[bi].reshape(4, 128, D)
                .transpose(1, 0, 2).reshape(128, 4 * D))
        in_maps.append({
            "axt8": np.ascontiguousarray(axt_p),
            "adj8": np.ascontiguousarray(adj_p),
            "awm": np.ascontiguousarray(awm_p),
            "negm": np.ascontiguousarray(negm_p),
            "axtb": np.ascontiguousarray(axtb_p),
            "xnb": np.ascontiguousarray(xnb_p),
            "W1s8": W1s8, "W2b": W2bb, "b1r16": b1r16, "b2r": b2rr,
            "b1B8": b1B8, "b2col": b2col, "mw": mw_p,
        })
    return in_maps, n_slots, order


def _assemble(results, order):
    out = np.empty((B, D), dtype=np.float32)
    for c in range(NCORES):
        outR = results[c]["outR"].reshape(BPC, D)
        sume = results[c]["sume"].reshape(-1)  # [BPC]
        for b in range(BPC):
            out[order[b * NCORES + c]] = outR[b] / sume[b]
    return out


def kernel(**inputs):
    from concourse.bass_utils import run_bass_kernel_spmd

    in_maps, n_slots, order = _plan(inputs)
    key = (BPC, n_slots)
    if key not in _nc_cache:
        _nc_cache[key] = _build_nc(BPC, n_slots)
    nc = _nc_cache[key]
    res = run_bass_kernel_spmd(nc, in_maps, list(range(NCORES)))
    return _assemble(res.results, order)


# revision 36
# speedup vs baseline: 1.1951x; 1.0053x over previous
"""ASGCN unit kernel for 8 Trainium2 NeuronCores (data-parallel over batch).

Contract: kernel(**inputs) takes the FULL unsharded inputs and returns the
FULL [128, 256] float32 output. Batch is sharded 16 samples/core across 8
cores; all parameters are replicated.

v2 design (evolved from the fp8 baseline after trace analysis):
  - position_weight, degree norm and fp8 scales are folded on host into the
    transposed adjacency (adjTw = adjT * 4096 * w[t] * dinv[s]) exactly as
    before; samples are sorted by n = ceil(text_len/128) into per-core slots
    sharing one slot->n pattern (SPMD) so matmuls skip structurally-zero
    128-chunks.
  - ALL inputs are shipped as a few large per-core packed DRAM blobs and
    loaded with ~18 big DMAs on the sync queue (the old per-sample DMA
    scheme kept the sync engine ~90% busy generating descriptors).
  - attention is restructured to be (almost) all-tensor:
      * logits computed TRANSPOSED: ps_lT[t,1] per 128-chunk via 8 tiny
        matmuls (lhsT = bf16 xT chunk, rhs = xs column),
      * exp on [128,4] (scalar engine, 128-partition utilization) with the
        per-partition accumulator collected into sumP[:, b],
      * weighted sum as 4 N=256 bf16 matmuls against a NORMAL-layout bf16
        copy of text_out (new input), giving the output row [1,256] in PSUM,
      * sum(exp) for all 16 samples reduced with ONE final f32 matmul
        (ones.T @ sumP).
    This removes the p-broadcast matmul, the [128,512] scalar copy and the
    two [128,512] vector accumulate-STTs per sample of the old design.
  - GCN layer 2 (window) runs fp8 for the adjacency contraction then bf16
    for the W2 matmul (better accuracy than the old all-fp8 path).
  - PSUM->SBUF epilogues are spread across scalar/vector; gpsimd (no PSUM
    port) takes the SBUF-only relu+cast work.
"""

import sys

if "/opt/trn_rl_repo" not in sys.path:
    sys.path.insert(0, "/opt/trn_rl_repo")

import numpy as np
import ml_dtypes

B, S, D, WIN = 128, 512, 256, 8
NCORES = 8
BPC = B // NCORES  # samples per core
BF = ml_dtypes.bfloat16
F8 = ml_dtypes.float8_e4m3  # TRN fp8e4: max +-240

_nc_cache = {}
USE_BIAS_MM = False
USE_FUSED_P3B = True


def _offsets(n_slots):
    """Per-slot element offsets (per partition) into the packed blobs."""
    axt_off, adj_off, awm_off = [0], [0], [0]
    for n in n_slots:
        axt_off.append(axt_off[-1] + 2 * 128 * n)      # [128, 2, 128n] fp8
        adj_off.append(adj_off[-1] + n * 128 * n)      # [128, n, 128n] fp8
        awm_off.append(awm_off[-1] + n * WIN)          # [128, n, WIN] fp8
    return axt_off, adj_off, awm_off


def _build_nc(bpc, n_slots):
    from contextlib import ExitStack

    import concourse.bass as bass
    import concourse.tile as tile
    from concourse import bacc, mybir

    dt = mybir.dt
    f32, bf16, f8 = dt.float32, dt.bfloat16, dt.float8e4
    AF = mybir.ActivationFunctionType
    OP = mybir.AluOpType
    DR = mybir.MatmulPerfMode.DoubleRow
    ts = bass.ts

    axt_off, adj_off, awm_off = _offsets(n_slots)

    nc = bacc.Bacc("TRN2", target_bir_lowering=False, debug=False,
                   num_devices=NCORES)

    # --- DRAM parameters: packed per-core blobs ---
    axt8_d = nc.declare_dram_parameter("axt8", [128, axt_off[-1]], f8,
                                       isOutput=False)
    adj8_d = nc.declare_dram_parameter("adj8", [128, adj_off[-1]], f8,
                                       isOutput=False)
    awm_d = nc.declare_dram_parameter("awm", [128, awm_off[-1]], f8,
                                      isOutput=False)
    negm_d = nc.declare_dram_parameter("negm", [1, bpc * WIN], bf16,
                                       isOutput=False)
    axtb_d = nc.declare_dram_parameter("axtb", [128, bpc * 2 * S], bf16,
                                       isOutput=False)
    xnb_d = nc.declare_dram_parameter("xnb", [128, bpc * 2 * S], bf16,
                                      isOutput=False)
    W1_d = nc.declare_dram_parameter("W1s8", [128, 2, D], f8, isOutput=False)
    W2_d = nc.declare_dram_parameter("W2b", [128, 2, D], bf16, isOutput=False)
    b1_d = nc.declare_dram_parameter("b1r16", [1, 2, D], bf16, isOutput=False)
    b2_d = nc.declare_dram_parameter("b2r", [1, 2, 128], bf16, isOutput=False)
    b1B_d = nc.declare_dram_parameter("b1B8", [128, 2, D], bf16, isOutput=False)
    b2c_d = nc.declare_dram_parameter("b2col", [128, 2], f32, isOutput=False)
    mw_d = nc.declare_dram_parameter("mw", [128, bpc * WIN], bf16, isOutput=False)
    outR_d = nc.declare_dram_parameter("outR", [1, bpc * D], f32,
                                       isOutput=True)
    sume_d = nc.declare_dram_parameter("sume", [1, bpc], f32, isOutput=True)

    LAG_P2, LAG_P3, LAG_P4, LAG_WS = 1, 2, 3, 3
    NSTEP = bpc + LAG_WS

    with tile.TileContext(nc) as tc, ExitStack() as ctx:
        const = ctx.enter_context(tc.tile_pool(name="const", bufs=1))
        pmid = ctx.enter_context(tc.tile_pool(name="pmid", bufs=6))
        psmall = ctx.enter_context(tc.tile_pool(name="psmall", bufs=8))
        pstage = ctx.enter_context(tc.tile_pool(name="pstage", bufs=1))
        psH = ctx.enter_context(tc.tile_pool(name="psH", bufs=2, space="PSUM"))
        psG = ctx.enter_context(tc.tile_pool(name="psG", bufs=2, space="PSUM"))
        psS = ctx.enter_context(tc.tile_pool(name="psS", bufs=1, space="PSUM"))
        psW = ctx.enter_context(tc.tile_pool(name="psW", bufs=1, space="PSUM"))
        # logits column [:, 0:4] and the ws output row [0:1, 4:260] share one
        # bank-sized tile (they are serially dependent through exp anyway)
        psLO = ctx.enter_context(tc.tile_pool(name="psLO", bufs=2,
                                              space="PSUM"))

        # ---- PE warmup: the HAM clock gate starts at K=4/8 (1.2 GHz) and
        # only opens after ~3.4us of sustained array activity. Burn dummy
        # dense DR matmuls during the initial DMA-wait window (tensor would
        # be idle anyway) so the real p1/p2 stream starts at 2.4 GHz. ----
        dumw = const.tile([128, 2, 128], f8, tag="dumw")
        nc.vector.memset(dumw[:], 1.0)
        ps_w = psW.tile([128, 128], f32, tag="ps_w")
        for _ in range(28):
            nc.tensor.matmul(ps_w[:], dumw[:, :, 0:128], dumw[:, :, 0:128],
                             perf_mode=DR)

        # ---- input SBUF blobs + the DMA schedule (sync queue only) ----
        W1s8 = const.tile([128, 2, D], f8, tag="W1s8")
        nc.sync.dma_start(W1s8[:], W1_d[:])

        # chunked blobs: independent tiles so readers only wait their chunk;
        # leading chunks are small so the pipeline starts ASAP
        AXT_CH = [(0, 2), (2, 6), (8, 8)]
        ADJ_CH = [(0, 2), (2, 2), (4, 4), (8, 4), (12, 4)]
        ABC, XNC = 4, 4  # slots per chunk
        axt_t, adj_t, axtb_t, xnb_t = {}, {}, {}, {}

        def dma_axt(c0, cnt):
            e0, e1 = axt_off[c0], axt_off[min(c0 + cnt, bpc)]
            t = const.tile([128, e1 - e0], f8, name=f"axt{c0}", tag=f"axt{c0}")
            nc.sync.dma_start(t[:], axt8_d[:, e0:e1])
            for b in range(c0, min(c0 + cnt, bpc)):
                axt_t[b] = (t, axt_off[b] - e0)

        def dma_adj(c0, cnt):
            e0, e1 = adj_off[c0], adj_off[min(c0 + cnt, bpc)]
            t = const.tile([128, e1 - e0], f8, name=f"adj{c0}", tag=f"adj{c0}")
            nc.sync.dma_start(t[:], adj8_d[:, e0:e1])
            for b in range(c0, min(c0 + cnt, bpc)):
                adj_t[b] = (t, adj_off[b] - e0)

        def dma_axtb(c0):
            e0, e1 = c0 * 2 * S, min(c0 + ABC, bpc) * 2 * S
            t = const.tile([128, e1 - e0], bf16, name=f"axb{c0}",
                           tag=f"axb{c0}")
            nc.sync.dma_start(t[:], axtb_d[:, e0:e1])
            for b in range(c0, min(c0 + ABC, bpc)):
                axtb_t[b] = (t, (b - c0) * 2 * S)

        def dma_xnb(c0):
            e0, e1 = c0 * 2 * S, min(c0 + XNC, bpc) * 2 * S
            t = const.tile([128, e1 - e0], bf16, name=f"xnb{c0}",
                           tag=f"xnb{c0}")
            nc.sync.dma_start(t[:], xnb_d[:, e0:e1])
            for b in range(c0, min(c0 + XNC, bpc)):
                xnb_t[b] = (t, (b - c0) * 2 * S)

        dma_axt(*AXT_CH[0])
        dma_adj(*ADJ_CH[0])
        b1r16 = const.tile([1, 2, D], bf16, tag="b1r16")
        nc.sync.dma_start(b1r16[:], b1_d[:])
        dma_axt(*AXT_CH[1])
        dma_adj(*ADJ_CH[1])
        W2b = const.tile([128, 2, D], bf16, tag="W2b")
        nc.sync.dma_start(W2b[:], W2_d[:])
        b2r = const.tile([1, 2, 128], bf16, tag="b2r")
        nc.sync.dma_start(b2r[:], b2_d[:])
        awm = const.tile([128, awm_off[-1]], f8, tag="awm")
        nc.sync.dma_start(awm[:], awm_d[:])
        negm = const.tile([1, bpc * WIN], bf16, tag="negm")
        nc.sync.dma_start(negm[:], negm_d[:])
        b1B8 = const.tile([128, 2, D], bf16, tag="b1B8")
        nc.sync.dma_start(b1B8[:], b1B_d[:])
        b2col = const.tile([128, 2], f32, tag="b2col")
        nc.sync.dma_start(b2col[:], b2c_d[:])
        mw = const.tile([128, bpc * WIN], bf16, tag="mw")
        nc.sync.dma_start(mw[:], mw_d[:])
        dma_adj(*ADJ_CH[2])
        dma_axtb(0)
        dma_axt(*AXT_CH[2])
        dma_adj(*ADJ_CH[3])
        dma_axtb(4)
        dma_xnb(0)
        dma_adj(*ADJ_CH[4])
        dma_axtb(8)
        dma_xnb(4)
        dma_axtb(12)
        dma_xnb(8)
        dma_xnb(12)

        onescol = const.tile([128, 1], f32, tag="onescol")
        nc.vector.memset(onescol[:], 1.0)
        onesrow = const.tile([1, 128], bf16, tag="onesrow")
        nc.vector.memset(onesrow[:], 1.0)
        sumP = pstage.tile([128, bpc], f32, tag="sumP")
        HB = bpc // 2
        outRa = pstage.tile([1, HB * D], f32, tag="outRa")
        outRb = pstage.tile([1, HB * D], f32, tag="outRb")
        sume = pstage.tile([1, bpc], f32, tag="sume")

        T = {b: {} for b in range(bpc)}

        def emit_p1(b):
            # h1[s,e] = x[s,:] @ W1 ; lhsT = fp8 xT slice, rhs = 16*W1.
            # PSUM = 16*h1 -> fp8 copy (scalar/vector alternating).
            n = n_slots[b]
            at, ao = axt_t[b]
            axt = at[:, ao:ao + 2 * 128 * n].rearrange(
                "p (c s) -> p c s", c=2)
            h1s8 = pmid.tile([128, 4, D], f8, name="h1s8", tag="h1s8")
            for sc in range(n):
                ps_h = psH.tile([128, D], f32, name="ps_h", tag="ps_h")
                nc.tensor.matmul(ps_h[:], axt[:, :, ts(sc, 128)],
                                 W1s8[:, :, :], perf_mode=DR)
                if sc % 2 == 0:
                    nc.scalar.copy(h1s8[:, sc, :], ps_h[:])
                else:
                    nc.vector.tensor_copy(h1s8[:, sc, :], ps_h[:])
            T[b]["h1s8"] = h1s8

        def emit_p2h(b, half):
            # g1 = b1 + adjTw.T @ h1 ; x2 = fp8(relu(256*g1))
            n = n_slots[b]
            if half >= (n + 1) // 2:
                return
            at, ao = adj_t[b]
            adjs = at[:, ao:ao + n * 128 * n].rearrange(
                "p (c s) -> p c s", c=n)
            h1s8 = T[b]["h1s8"]
            if half == 0:
                x2 = pmid.tile([128, 4, D], f8, name="x2", tag="x2")
                T[b]["x2"] = x2
            else:
                x2 = T[b]["x2"]
            if True:
                w_ = min(2, n - 2 * half)
                ps_g = psG.tile([128, 2, D], f32, name="ps_g", tag="ps_g")
                if USE_BIAS_MM:
                    # bias first: PSUM = 65536*b1 via a K=1 bf16 matmul
                    nc.tensor.matmul(ps_g[:, 0:w_, :], onesrow[:],
                                     b1r16[:, 0:w_, :], start=True, stop=False)
                for sci in range(w_):
                    sc = 2 * half + sci
                    # DoubleRow over t-chunk pairs (fp8: 2 k-tiles/inst);
                    # each sci slice is its own open/close psum group
                    for tp in range(n // 2):
                        nc.tensor.matmul(
                            ps_g[:, sci, :],
                            adjs[:, 2 * tp:2 * tp + 2, ts(sc, 128)],
                            h1s8[:, 2 * tp:2 * tp + 2, :],
                            perf_mode=DR,
                            start=(not USE_BIAS_MM and tp == 0),
                            stop=(n % 2 == 0 and tp == n // 2 - 1))
                    if n % 2:
                        nc.tensor.matmul(
                            ps_g[:, sci, :],
                            adjs[:, n - 1, ts(sc, 128)],
                            h1s8[:, n - 1, :],
                            start=False,
                            stop=True)
                if USE_BIAS_MM:
                    # x2 = fp8(relu(2^-8 * PSUM)) straight out of PSUM
                    if half == 0:
                        nc.scalar.activation(
                            x2[:, 0:w_, :], ps_g[:, 0:w_, :], AF.Relu,
                            scale=1.0 / 256.0)
                    else:
                        nc.vector.tensor_scalar(
                            x2[:, 2:2 + w_, :], ps_g[:, 0:w_, :],
                            1.0 / 256.0, 0.0, op0=OP.mult, op1=OP.max)
                else:
                    gt = pmid.tile([128, 2, D], bf16, name="gt", tag="gt")
                    nc.vector.scalar_tensor_tensor(
                        gt[:, 0:w_, :], ps_g[:, 0:w_, :], 1.0 / 256.0,
                        b1B8[:, 0:w_, :], op0=OP.mult, op1=OP.add)
                    if half == 0:
                        nc.scalar.activation(
                            x2[:, 0:w_, :], gt[:, 0:w_, :], AF.Relu)
                    else:
                        nc.vector.tensor_scalar(
                            x2[:, 2:2 + w_, :], gt[:, 0:w_, :],
                            1.0, 0.0, op0=OP.mult, op1=OP.max)

        def emit_p3a(b):
            # window layer: ps_y = (256 x2).T @ (4096 awm) = 2^20 yT
            n = n_slots[b]
            x2 = T[b]["x2"]
            awms = awm[:, awm_off[b]:awm_off[b + 1]].rearrange(
                "p (c w) -> p c w", c=n)
            ps_y = psS.tile([128, 2, WIN], f32, name="ps_y", tag="ps_s")
            for dc in range(2):
                for sp in range(n // 2):
                    nc.tensor.matmul(ps_y[:, dc, :],
                                     x2[:, 2 * sp:2 * sp + 2, ts(dc, 128)],
                                     awms[:, 2 * sp:2 * sp + 2, :],
                                     perf_mode=DR,
                                     start=(sp == 0),
                                     stop=(n % 2 == 0 and sp == n // 2 - 1))
                if n % 2:
                    nc.tensor.matmul(ps_y[:, dc, :],
                                     x2[:, n - 1, ts(dc, 128)],
                                     awms[:, n - 1, :],
                                     start=False, stop=True)
            yTb = psmall.tile([128, 2, WIN], bf16, name="yTb", tag="yTb")
            nc.vector.tensor_scalar(yTb[:], ps_y[:], 2.0 ** -20, 0.0,
                                    op0=OP.mult, op1=OP.add)
            T[b]["yTb"] = yTb

        def emit_p3b(b):
            # ps_z = W2b.T @ yTb = z ; r1 = relu(z + b2) ;
            # xs = sum_w r1*mw -> xsb bf16 [128, 2]
            yTb = T[b]["yTb"]
            ps_z = psS.tile([128, 2, WIN], f32, name="ps_z", tag="ps_s")
            for ec in range(2):
                for dc in range(2):
                    nc.tensor.matmul(ps_z[:, ec, :],
                                     W2b[:, dc, ts(ec, 128)],
                                     yTb[:, dc, :],
                                     start=(dc == 0),
                                     stop=(not USE_FUSED_P3B and dc == 1))
                if USE_FUSED_P3B:
                    # + b2[e] (rank-1: b2 slice as weights x ones) and
                    # -1000 on masked window cols: relu also applies the mask
                    nc.tensor.matmul(ps_z[:, ec, :], b2r[:, ec, :],
                                     onesrow[0:1, 0:WIN],
                                     start=False, stop=False)
                    nc.tensor.matmul(ps_z[:, ec, :], onesrow[:],
                                     negm[:, b * WIN:(b + 1) * WIN],
                                     start=False, stop=True)
            xsb = psmall.tile([128, 2], bf16, name="xsb", tag="xsb")
            if USE_FUSED_P3B:
                for ec in range(2):
                    x3 = psmall.tile([128, WIN], f32, name="x3", tag="x3")
                    xs_f = psmall.tile([128, 1], f32, name="xs_f", tag="xs_f")
                    # out = relu(ps_z) via op0=max; op1 is the REDUCTION op
                    # when accum_out is set: accum = sum(out)
                    nc.vector.tensor_scalar(
                        x3[:], ps_z[:, ec, :], 0.0, None,
                        op0=OP.max, op1=OP.add, accum_out=xs_f[:])
                    nc.gpsimd.tensor_copy(xsb[:, ec:ec + 1], xs_f[:])
            else:
                r1 = psmall.tile([128, 2, WIN], f32, name="r1", tag="r1")
                for ec in range(2):
                    nc.scalar.activation(r1[:, ec, :], ps_z[:, ec, :],
                                         AF.Relu, bias=b2col[:, ec:ec + 1])
                for ec in range(2):
                    x3 = psmall.tile([128, WIN], f32, name="x3", tag="x3")
                    xs_f = psmall.tile([128, 1], f32, name="xs_f", tag="xs_f")
                    nc.vector.scalar_tensor_tensor(
                        x3[:], r1[:, ec, :], 1.0,
                        mw[:, b * WIN:(b + 1) * WIN],
                        op0=OP.mult, op1=OP.mult, accum_out=xs_f[:])
                    nc.gpsimd.tensor_copy(xsb[:, ec:ec + 1], xs_f[:])
            T[b]["xsb"] = xsb

        def emit_p4l(b):
            # transposed logits: ps_lT[:, tc] = sum_dc axtb[:,dc,tc*128:].T @ xs
            at, ao = axtb_t[b]
            axtbs = at[:, ao:ao + 2 * S].rearrange("p (c s) -> p c s", c=2)
            xsb = T[b]["xsb"]
            lo = psLO.tile([128, 260], f32, name="ps_lo", tag="ps_lo")
            for tc_ in range(4):
                for dc in range(2):
                    nc.tensor.matmul(lo[:, tc_:tc_ + 1],
                                     axtbs[:, dc, ts(tc_, 128)],
                                     xsb[:, dc:dc + 1],
                                     start=(dc == 0), stop=(dc == 1))
            pexp = psmall.tile([128, 4], bf16, name="pexp", tag="pexp")
            nc.scalar.activation(pexp[:], lo[:, 0:4], AF.Exp,
                                 accum_out=sumP[:, b:b + 1])
            T[b]["pexp"] = pexp
            T[b]["lo"] = lo

        def emit_ws(b):
            # weighted sum: out[1, D] = sum_tc pexp[:, tc].T @ xnb[:, tc, :]
            at, ao = xnb_t[b]
            xns = at[:, ao:ao + 2 * S].rearrange("p (c s) -> p c s", c=4)
            pexp = T[b]["pexp"]
            lo = T[b]["lo"]
            for tc_ in range(4):
                nc.tensor.matmul(lo[0:1, 4:4 + D], pexp[:, tc_:tc_ + 1],
                                 xns[:, tc_, :],
                                 start=(tc_ == 0), stop=(tc_ == 3))
            outR = outRa if b < HB else outRb
            bo = b if b < HB else b - HB
            if b % 2 == 0:
                nc.scalar.copy(outR[:, bo * D:(bo + 1) * D], lo[0:1, 4:4 + D])
            else:
                nc.vector.tensor_copy(outR[:, bo * D:(bo + 1) * D],
                                      lo[0:1, 4:4 + D])

        for i in range(NSTEP):
            if 0 <= i - LAG_P4 < bpc:
                emit_p4l(i - LAG_P4)
            if i < bpc:
                emit_p1(i)
            if 0 <= i - LAG_P3 < bpc:
                emit_p3a(i - LAG_P3)
            if 0 <= i - LAG_P2 < bpc:
                emit_p2h(i - LAG_P2, 0)
            if 0 <= i - LAG_P3 < bpc:
                emit_p3b(i - LAG_P3)
            if 0 <= i - LAG_P2 < bpc:
                emit_p2h(i - LAG_P2, 1)
            if 0 <= i - LAG_WS < bpc:
                emit_ws(i - LAG_WS)
            if i - LAG_WS == HB - 1:
                # first-half output rows are final: overlap their store
                nc.sync.dma_start(outR_d[:, 0:HB * D], outRa[:])

        # sume[1, bpc] = ones.T @ sumP  (single f32 matmul)
        ps_s = psLO.tile([128, 260], f32, name="ps_sm", tag="ps_lo")
        nc.tensor.matmul(ps_s[0:1, 0:bpc], onescol[:], sumP[:])
        nc.scalar.copy(sume[:], ps_s[0:1, 0:bpc])

        nc.sync.dma_start(sume_d[:], sume[:])
        nc.sync.dma_start(outR_d[:, HB * D:], outRb[:])

    nc.compile()
    return nc


def _f8(x):
    return np.clip(x, -240.0, 240.0).astype(F8)


def _plan(inputs):
    """Host-side preprocessing: fold position weight / degree norm / fp8
    scales; sort samples by chunk count into per-core slots; pack per-core
    DRAM blobs. order[b*NCORES + c] is the original sample index placed in
    slot b of core c."""
    text_out = np.asarray(inputs["text_out"], dtype=np.float32)
    adj = np.asarray(inputs["adj"], dtype=np.float32)
    W1 = np.asarray(inputs["W1"], dtype=np.float32)
    b1 = np.asarray(inputs["b1"], dtype=np.float32)
    W2 = np.asarray(inputs["W2"], dtype=np.float32)
    b2 = np.asarray(inputs["b2"], dtype=np.float32)
    tl = np.asarray(inputs["text_len"]).astype(np.int64)
    al = np.asarray(inputs["aspect_len"]).astype(np.int64)
    ll = np.asarray(inputs["left_len"]).astype(np.int64)

    n_all = np.minimum(4, np.maximum(2, (tl + 127) // 128)).astype(np.int64)
    # descending: big-n slots first (denser warmup, lighter pipeline drain)
    order = np.argsort(-n_all, kind="stable")       # [B]
    n_slots = tuple(int(n_all[order[b * NCORES:(b + 1) * NCORES]].max())
                    for b in range(BPC))
    axt_off, adj_off, awm_off = _offsets(n_slots)

    j = np.arange(S)[None, :]
    start = ll[:, None]
    end = (ll + al - 1)[:, None]
    ctxlen = (tl - al).astype(np.float32)[:, None]
    w = np.where(j < start, 1.0 - (start - j) / ctxlen,
                 np.where(j <= end, 0.0,
                          np.where(j < tl[:, None], 1.0 - (j - end) / ctxlen,
                                   0.0))).astype(np.float32)      # [B,S]
    dinv = (1.0 / (adj.sum(axis=2) + 1.0)).astype(np.float32)     # [B,S]

    # transposed adjacency, position weight (t) and 1/den (s) folded, *4096
    adjTw = adj.transpose(0, 2, 1) * (4096.0 * w[:, :, None]) * dinv[:, None, :]
    adj8f = _f8(adjTw)                  # [B, t(S), s(S)]
    xT = text_out.transpose(0, 2, 1)    # [B, D, S]
    axt8f = _f8(xT)
    axtbf = xT.astype(BF)
    xnbf = text_out.astype(BF)          # [B, S(t), D]

    win = np.clip(ll[:, None] + np.arange(WIN)[None, :], 0, S - 1)  # [B,WIN]
    adj_win = np.take_along_axis(adj, win[:, :, None], axis=1)      # [B,WIN,S]
    dinvW = np.take_along_axis(dinv, win, axis=1)                   # [B,WIN]
    adjWTw = (adj_win.transpose(0, 2, 1) * (4096.0 * w[:, :, None])
              * dinvW[:, None, :])
    awm8f = _f8(adjWTw)                 # [B, s(S), WIN]

    # 0 on active window cols, -1000 on masked: relu applies the mask
    negmf = np.where(np.arange(WIN)[None, :] < al[:, None],
                     0.0, -1000.0).astype(BF)       # [B, WIN]

    W1s8 = _f8(np.ascontiguousarray(
        (16.0 * W1).reshape(2, 128, D).transpose(1, 0, 2)))
    W2bb = np.ascontiguousarray(
        W2.reshape(2, 128, D).transpose(1, 0, 2)).astype(BF)
    b1r16 = np.ascontiguousarray(
        (65536.0 * np.tile(b1, 2)).reshape(1, 2, D)).astype(BF)
    b2rr = np.ascontiguousarray(b2.reshape(1, 2, 128)).astype(BF)
    b1B8 = np.ascontiguousarray(np.broadcast_to(
        (256.0 * np.tile(b1, 2)).reshape(1, 2, D), (128, 2, D))).astype(BF)
    b2col = np.ascontiguousarray(b2.reshape(2, 128).T).astype(np.float32)

    in_maps = []
    for c in range(NCORES):
        idx = order[np.arange(BPC) * NCORES + c]   # slot b -> order[b*8+c]
        axt_p = np.empty((128, axt_off[-1]), dtype=F8)
        adj_p = np.empty((128, adj_off[-1]), dtype=F8)
        awm_p = np.empty((128, awm_off[-1]), dtype=F8)
        negm_p = np.empty((1, BPC * WIN), dtype=BF)
        mw_p = np.empty((128, BPC * WIN), dtype=BF)
        axtb_p = np.empty((128, BPC * 2 * S), dtype=BF)
        xnb_p = np.empty((128, BPC * 2 * S), dtype=BF)
        for b in range(BPC):
            bi = idx[b]
            n = n_slots[b]
            sa = 128 * n
            # axt8: [D, S] -> [128, 2(dc), 128n] -> flat
            axt_p[:, axt_off[b]:axt_off[b + 1]] = (
                axt8f[bi].reshape(2, 128, S)[:, :, :sa]
                .transpose(1, 0, 2).reshape(128, 2 * sa))
            # adj8: [t(S), s(S)] -> [128(t in chunk), n(tc), 128n(s)]
            adj_p[:, adj_off[b]:adj_off[b + 1]] = (
                adj8f[bi].reshape(4, 128, S)[:n, :, :sa]
                .transpose(1, 0, 2).reshape(128, n * sa))
            # awm: [s(S), WIN] -> [128, n(sc), WIN]
            awm_p[:, awm_off[b]:awm_off[b + 1]] = (
                awm8f[bi].reshape(4, 128, WIN)[:n]
                .transpose(1, 0, 2).reshape(128, n * WIN))
            negm_p[0, b * WIN:(b + 1) * WIN] = negmf[bi]
            mw_p[:, b * WIN:(b + 1) * WIN] = (negmf[bi] == 0).astype(BF)[None, :]
            # axtb: [D, S] -> [128, 2(dc), S]
            axtb_p[:, b * 2 * S:(b + 1) * 2 * S] = (
                axtbf[bi].reshape(2, 128, S)
                .transpose(1, 0, 2).reshape(128, 2 * S))
            # xnb: [S(t), D] -> [128(t in chunk), 4(tc), D]
            xnb_p[:, b * 2 * S:(b + 1) * 2 * S] = (
                xnbf[bi].reshape(4, 128, D)
                .transpose(1, 0, 2).reshape(128, 4 * D))
        in_maps.append({
            "axt8": np.ascontiguousarray(axt_p),
            "adj8": np.ascontiguousarray(adj_p),
            "awm": np.ascontiguousarray(awm_p),
            "negm": np.ascontiguousarray(negm_p),
            "axtb": np.ascontiguousarray(axtb_p),
            "xnb": np.ascontiguousarray(xnb_p),
            "W1s8": W1s8, "W2b": W2bb, "b1r16": b1r16, "b2r": b2rr,
            "b1B8": b1B8, "b2col": b2col, "mw": mw_p,
        })
    return in_maps, n_slots, order


def _assemble(results, order):
    out = np.empty((B, D), dtype=np.float32)
    for c in range(NCORES):
        outR = results[c]["outR"].reshape(BPC, D)
        sume = results[c]["sume"].reshape(-1)  # [BPC]
        for b in range(BPC):
            out[order[b * NCORES + c]] = outR[b] / sume[b]
    return out


def kernel(**inputs):
    from concourse.bass_utils import run_bass_kernel_spmd

    in_maps, n_slots, order = _plan(inputs)
    key = (BPC, n_slots)
    if key not in _nc_cache:
        _nc_cache[key] = _build_nc(BPC, n_slots)
    nc = _nc_cache[key]
    res = run_bass_kernel_spmd(nc, in_maps, list(range(NCORES)))
    return _assemble(res.results, order)


# revision 38
# speedup vs baseline: 1.2165x; 1.0180x over previous
"""ASGCN unit kernel for 8 Trainium2 NeuronCores (data-parallel over batch).

Contract: kernel(**inputs) takes the FULL unsharded inputs and returns the
FULL [128, 256] float32 output. Batch is sharded 16 samples/core across 8
cores; all parameters are replicated.

v2 design (evolved from the fp8 baseline after trace analysis):
  - position_weight, degree norm and fp8 scales are folded on host into the
    transposed adjacency (adjTw = adjT * 4096 * w[t] * dinv[s]) exactly as
    before; samples are sorted by n = ceil(text_len/128) into per-core slots
    sharing one slot->n pattern (SPMD) so matmuls skip structurally-zero
    128-chunks.
  - ALL inputs are shipped as a few large per-core packed DRAM blobs and
    loaded with ~18 big DMAs on the sync queue (the old per-sample DMA
    scheme kept the sync engine ~90% busy generating descriptors).
  - attention is restructured to be (almost) all-tensor:
      * logits computed TRANSPOSED: ps_lT[t,1] per 128-chunk via 8 tiny
        matmuls (lhsT = bf16 xT chunk, rhs = xs column),
      * exp on [128,4] (scalar engine, 128-partition utilization) with the
        per-partition accumulator collected into sumP[:, b],
      * weighted sum as 4 N=256 bf16 matmuls against a NORMAL-layout bf16
        copy of text_out (new input), giving the output row [1,256] in PSUM,
      * sum(exp) for all 16 samples reduced with ONE final f32 matmul
        (ones.T @ sumP).
    This removes the p-broadcast matmul, the [128,512] scalar copy and the
    two [128,512] vector accumulate-STTs per sample of the old design.
  - GCN layer 2 (window) runs fp8 for the adjacency contraction then bf16
    for the W2 matmul (better accuracy than the old all-fp8 path).
  - PSUM->SBUF epilogues are spread across scalar/vector; gpsimd (no PSUM
    port) takes the SBUF-only relu+cast work.
"""

import sys

if "/opt/trn_rl_repo" not in sys.path:
    sys.path.insert(0, "/opt/trn_rl_repo")

import numpy as np
import ml_dtypes

B, S, D, WIN = 128, 512, 256, 8
NCORES = 8
BPC = B // NCORES  # samples per core
BF = ml_dtypes.bfloat16
F8 = ml_dtypes.float8_e4m3  # TRN fp8e4: max +-240

_nc_cache = {}
USE_BIAS_MM = False
USE_FUSED_P3B = True


def _offsets(n_slots):
    """Per-slot element offsets (per partition) into the packed blobs."""
    axt_off, adj_off, awm_off = [0], [0], [0]
    for n in n_slots:
        axt_off.append(axt_off[-1] + 2 * 128 * n)      # [128, 2, 128n] fp8
        adj_off.append(adj_off[-1] + n * 128 * n)      # [128, n, 128n] fp8
        awm_off.append(awm_off[-1] + n * WIN)          # [128, n, WIN] fp8
    return axt_off, adj_off, awm_off


def _build_nc(bpc, n_slots):
    from contextlib import ExitStack

    import concourse.bass as bass
    import concourse.tile as tile
    from concourse import bacc, mybir

    dt = mybir.dt
    f32, bf16, f8 = dt.float32, dt.bfloat16, dt.float8e4
    AF = mybir.ActivationFunctionType
    OP = mybir.AluOpType
    DR = mybir.MatmulPerfMode.DoubleRow
    ts = bass.ts

    axt_off, adj_off, awm_off = _offsets(n_slots)

    nc = bacc.Bacc("TRN2", target_bir_lowering=False, debug=False,
                   num_devices=NCORES)

    # --- DRAM parameters: packed per-core blobs ---
    axt8_d = nc.declare_dram_parameter("axt8", [128, axt_off[-1]], f8,
                                       isOutput=False)
    adj8_d = nc.declare_dram_parameter("adj8", [128, adj_off[-1]], f8,
                                       isOutput=False)
    awm_d = nc.declare_dram_parameter("awm", [128, awm_off[-1]], f8,
                                      isOutput=False)
    negm_d = nc.declare_dram_parameter("negm", [1, bpc * WIN], bf16,
                                       isOutput=False)
    axtb_d = nc.declare_dram_parameter("axtb", [128, bpc * 2 * S], bf16,
                                       isOutput=False)
    xnb_d = nc.declare_dram_parameter("xnb", [128, bpc * 2 * S], bf16,
                                      isOutput=False)
    W1_d = nc.declare_dram_parameter("W1s8", [128, 2, D], f8, isOutput=False)
    W2_d = nc.declare_dram_parameter("W2b", [128, 2, D], bf16, isOutput=False)
    b1_d = nc.declare_dram_parameter("b1r16", [1, 2, D], bf16, isOutput=False)
    b2_d = nc.declare_dram_parameter("b2r", [1, 2, 128], bf16, isOutput=False)
    b1B_d = nc.declare_dram_parameter("b1B8", [128, 2, D], bf16, isOutput=False)
    b2c_d = nc.declare_dram_parameter("b2col", [128, 2], f32, isOutput=False)
    mw_d = nc.declare_dram_parameter("mw", [128, bpc * WIN], bf16, isOutput=False)
    outR_d = nc.declare_dram_parameter("outR", [1, bpc * D], f32,
                                       isOutput=True)
    sume_d = nc.declare_dram_parameter("sume", [1, bpc], f32, isOutput=True)

    LAG_P2, LAG_P3, LAG_P4, LAG_WS = 1, 2, 3, 4
    NSTEP = bpc + LAG_WS

    with tile.TileContext(nc) as tc, ExitStack() as ctx:
        const = ctx.enter_context(tc.tile_pool(name="const", bufs=1))
        pmid = ctx.enter_context(tc.tile_pool(name="pmid", bufs=6))
        psmall = ctx.enter_context(tc.tile_pool(name="psmall", bufs=8))
        pstage = ctx.enter_context(tc.tile_pool(name="pstage", bufs=1))
        psH = ctx.enter_context(tc.tile_pool(name="psH", bufs=2, space="PSUM"))
        psG = ctx.enter_context(tc.tile_pool(name="psG", bufs=2, space="PSUM"))
        psS = ctx.enter_context(tc.tile_pool(name="psS", bufs=1, space="PSUM"))
        psW = ctx.enter_context(tc.tile_pool(name="psW", bufs=1, space="PSUM"))
        # logits column [:, 0:4] and the ws output row [0:1, 4:260] share one
        # bank-sized tile (they are serially dependent through exp anyway)
        psLO = ctx.enter_context(tc.tile_pool(name="psLO", bufs=2,
                                              space="PSUM"))

        # ---- PE warmup: the HAM clock gate starts at K=4/8 (1.2 GHz) and
        # only opens after ~3.4us of sustained array activity. Burn dummy
        # dense DR matmuls during the initial DMA-wait window (tensor would
        # be idle anyway) so the real p1/p2 stream starts at 2.4 GHz. ----
        dumw = const.tile([128, 2, 128], f8, tag="dumw")
        nc.vector.memset(dumw[:], 1.0)
        ps_w = psW.tile([128, 128], f32, tag="ps_w")
        for _ in range(28):
            nc.tensor.matmul(ps_w[:], dumw[:, :, 0:128], dumw[:, :, 0:128],
                             perf_mode=DR)

        # ---- input SBUF blobs + the DMA schedule (sync queue only) ----
        W1s8 = const.tile([128, 2, D], f8, tag="W1s8")
        nc.sync.dma_start(W1s8[:], W1_d[:])

        # chunked blobs: independent tiles so readers only wait their chunk;
        # leading chunks are small so the pipeline starts ASAP
        AXT_CH = [(0, 2), (2, 6), (8, 8)]
        ADJ_CH = [(0, 2), (2, 2), (4, 4), (8, 4), (12, 4)]
        ABC, XNC = 4, 4  # slots per chunk
        axt_t, adj_t, axtb_t, xnb_t = {}, {}, {}, {}

        def dma_axt(c0, cnt):
            e0, e1 = axt_off[c0], axt_off[min(c0 + cnt, bpc)]
            t = const.tile([128, e1 - e0], f8, name=f"axt{c0}", tag=f"axt{c0}")
            nc.sync.dma_start(t[:], axt8_d[:, e0:e1])
            for b in range(c0, min(c0 + cnt, bpc)):
                axt_t[b] = (t, axt_off[b] - e0)

        def dma_adj(c0, cnt):
            e0, e1 = adj_off[c0], adj_off[min(c0 + cnt, bpc)]
            t = const.tile([128, e1 - e0], f8, name=f"adj{c0}", tag=f"adj{c0}")
            nc.sync.dma_start(t[:], adj8_d[:, e0:e1])
            for b in range(c0, min(c0 + cnt, bpc)):
                adj_t[b] = (t, adj_off[b] - e0)

        def dma_axtb(c0):
            e0, e1 = c0 * 2 * S, min(c0 + ABC, bpc) * 2 * S
            t = const.tile([128, e1 - e0], bf16, name=f"axb{c0}",
                           tag=f"axb{c0}")
            nc.sync.dma_start(t[:], axtb_d[:, e0:e1])
            for b in range(c0, min(c0 + ABC, bpc)):
                axtb_t[b] = (t, (b - c0) * 2 * S)

        def dma_xnb(c0):
            e0, e1 = c0 * 2 * S, min(c0 + XNC, bpc) * 2 * S
            t = const.tile([128, e1 - e0], bf16, name=f"xnb{c0}",
                           tag=f"xnb{c0}")
            nc.sync.dma_start(t[:], xnb_d[:, e0:e1])
            for b in range(c0, min(c0 + XNC, bpc)):
                xnb_t[b] = (t, (b - c0) * 2 * S)

        dma_axt(*AXT_CH[0])
        dma_adj(*ADJ_CH[0])
        b1r16 = const.tile([1, 2, D], bf16, tag="b1r16")
        nc.sync.dma_start(b1r16[:], b1_d[:])
        dma_axt(*AXT_CH[1])
        dma_adj(*ADJ_CH[1])
        W2b = const.tile([128, 2, D], bf16, tag="W2b")
        nc.sync.dma_start(W2b[:], W2_d[:])
        b2r = const.tile([1, 2, 128], bf16, tag="b2r")
        nc.sync.dma_start(b2r[:], b2_d[:])
        awm = const.tile([128, awm_off[-1]], f8, tag="awm")
        nc.sync.dma_start(awm[:], awm_d[:])
        negm = const.tile([1, bpc * WIN], bf16, tag="negm")
        nc.sync.dma_start(negm[:], negm_d[:])
        b1B8 = const.tile([128, 2, D], bf16, tag="b1B8")
        nc.sync.dma_start(b1B8[:], b1B_d[:])
        b2col = const.tile([128, 2], f32, tag="b2col")
        nc.sync.dma_start(b2col[:], b2c_d[:])
        mw = const.tile([128, bpc * WIN], bf16, tag="mw")
        nc.sync.dma_start(mw[:], mw_d[:])
        dma_adj(*ADJ_CH[2])
        dma_axtb(0)
        dma_axt(*AXT_CH[2])
        dma_adj(*ADJ_CH[3])
        dma_axtb(4)
        dma_xnb(0)
        dma_adj(*ADJ_CH[4])
        dma_axtb(8)
        dma_xnb(4)
        dma_axtb(12)
        dma_xnb(8)
        dma_xnb(12)

        onescol = const.tile([128, 1], f32, tag="onescol")
        nc.vector.memset(onescol[:], 1.0)
        onesrow = const.tile([1, 128], bf16, tag="onesrow")
        nc.vector.memset(onesrow[:], 1.0)
        sumP = pstage.tile([128, bpc], f32, tag="sumP")
        HB = bpc // 2
        outRa = pstage.tile([1, HB * D], f32, tag="outRa")
        outRb = pstage.tile([1, HB * D], f32, tag="outRb")
        sume = pstage.tile([1, bpc], f32, tag="sume")

        T = {b: {} for b in range(bpc)}

        def emit_p1(b):
            # h1[s,e] = x[s,:] @ W1 ; lhsT = fp8 xT slice, rhs = 16*W1.
            # PSUM = 16*h1 -> fp8 copy (scalar/vector alternating).
            n = n_slots[b]
            at, ao = axt_t[b]
            axt = at[:, ao:ao + 2 * 128 * n].rearrange(
                "p (c s) -> p c s", c=2)
            h1s8 = pmid.tile([128, 4, D], f8, name="h1s8", tag="h1s8")
            for sc in range(n):
                ps_h = psH.tile([128, D], f32, name="ps_h", tag="ps_h")
                nc.tensor.matmul(ps_h[:], axt[:, :, ts(sc, 128)],
                                 W1s8[:, :, :], perf_mode=DR)
                if sc % 2 == 0:
                    nc.scalar.copy(h1s8[:, sc, :], ps_h[:])
                else:
                    nc.vector.tensor_copy(h1s8[:, sc, :], ps_h[:])
            T[b]["h1s8"] = h1s8

        def emit_p2h(b, half):
            # g1 = b1 + adjTw.T @ h1 ; x2 = fp8(relu(256*g1))
            n = n_slots[b]
            if half >= (n + 1) // 2:
                return
            at, ao = adj_t[b]
            adjs = at[:, ao:ao + n * 128 * n].rearrange(
                "p (c s) -> p c s", c=n)
            h1s8 = T[b]["h1s8"]
            if half == 0:
                x2 = pmid.tile([128, 4, D], f8, name="x2", tag="x2")
                T[b]["x2"] = x2
            else:
                x2 = T[b]["x2"]
            if True:
                w_ = min(2, n - 2 * half)
                ps_g = psG.tile([128, 2, D], f32, name="ps_g", tag="ps_g")
                if USE_BIAS_MM:
                    # bias first: PSUM = 65536*b1 via a K=1 bf16 matmul
                    nc.tensor.matmul(ps_g[:, 0:w_, :], onesrow[:],
                                     b1r16[:, 0:w_, :], start=True, stop=False)
                for sci in range(w_):
                    sc = 2 * half + sci
                    # DoubleRow over t-chunk pairs (fp8: 2 k-tiles/inst);
                    # each sci slice is its own open/close psum group
                    for tp in range(n // 2):
                        nc.tensor.matmul(
                            ps_g[:, sci, :],
                            adjs[:, 2 * tp:2 * tp + 2, ts(sc, 128)],
                            h1s8[:, 2 * tp:2 * tp + 2, :],
                            perf_mode=DR,
                            start=(not USE_BIAS_MM and tp == 0),
                            stop=(n % 2 == 0 and tp == n // 2 - 1))
                    if n % 2:
                        nc.tensor.matmul(
                            ps_g[:, sci, :],
                            adjs[:, n - 1, ts(sc, 128)],
                            h1s8[:, n - 1, :],
                            start=False,
                            stop=True)
                if USE_BIAS_MM:
                    # x2 = fp8(relu(2^-8 * PSUM)) straight out of PSUM
                    if half == 0:
                        nc.scalar.activation(
                            x2[:, 0:w_, :], ps_g[:, 0:w_, :], AF.Relu,
                            scale=1.0 / 256.0)
                    else:
                        nc.vector.tensor_scalar(
                            x2[:, 2:2 + w_, :], ps_g[:, 0:w_, :],
                            1.0 / 256.0, 0.0, op0=OP.mult, op1=OP.max)
                else:
                    gt = pmid.tile([128, 2, D], bf16, name="gt", tag="gt")
                    nc.vector.scalar_tensor_tensor(
                        gt[:, 0:w_, :], ps_g[:, 0:w_, :], 1.0 / 256.0,
                        b1B8[:, 0:w_, :], op0=OP.mult, op1=OP.add)
                    if half == 0:
                        nc.scalar.activation(
                            x2[:, 0:w_, :], gt[:, 0:w_, :], AF.Relu)
                    else:
                        nc.vector.tensor_scalar(
                            x2[:, 2:2 + w_, :], gt[:, 0:w_, :],
                            1.0, 0.0, op0=OP.mult, op1=OP.max)

        def emit_p3a(b):
            # window layer: ps_y = (256 x2).T @ (4096 awm) = 2^20 yT
            n = n_slots[b]
            x2 = T[b]["x2"]
            awms = awm[:, awm_off[b]:awm_off[b + 1]].rearrange(
                "p (c w) -> p c w", c=n)
            ps_y = psS.tile([128, 2, WIN], f32, name="ps_y", tag="ps_s")
            for dc in range(2):
                for sp in range(n // 2):
                    nc.tensor.matmul(ps_y[:, dc, :],
                                     x2[:, 2 * sp:2 * sp + 2, ts(dc, 128)],
                                     awms[:, 2 * sp:2 * sp + 2, :],
                                     perf_mode=DR,
                                     start=(sp == 0),
                                     stop=(n % 2 == 0 and sp == n // 2 - 1))
                if n % 2:
                    nc.tensor.matmul(ps_y[:, dc, :],
                                     x2[:, n - 1, ts(dc, 128)],
                                     awms[:, n - 1, :],
                                     start=False, stop=True)
            yTb = psmall.tile([128, 2, WIN], bf16, name="yTb", tag="yTb")
            nc.vector.tensor_scalar(yTb[:], ps_y[:], 2.0 ** -20, 0.0,
                                    op0=OP.mult, op1=OP.add)
            T[b]["yTb"] = yTb

        def emit_p3b(b):
            # ps_z = W2b.T @ yTb = z ; r1 = relu(z + b2) ;
            # xs = sum_w r1*mw -> xsb bf16 [128, 2]
            yTb = T[b]["yTb"]
            ps_z = psS.tile([128, 2, WIN], f32, name="ps_z", tag="ps_s")
            for ec in range(2):
                for dc in range(2):
                    nc.tensor.matmul(ps_z[:, ec, :],
                                     W2b[:, dc, ts(ec, 128)],
                                     yTb[:, dc, :],
                                     start=(dc == 0),
                                     stop=(not USE_FUSED_P3B and dc == 1))
                if USE_FUSED_P3B:
                    # + b2[e] (rank-1: b2 slice as weights x ones) and
                    # -1000 on masked window cols: relu also applies the mask
                    nc.tensor.matmul(ps_z[:, ec, :], b2r[:, ec, :],
                                     onesrow[0:1, 0:WIN],
                                     start=False, stop=False)
                    nc.tensor.matmul(ps_z[:, ec, :], onesrow[:],
                                     negm[:, b * WIN:(b + 1) * WIN],
                                     start=False, stop=True)
            xsb = psmall.tile([128, 2], bf16, name="xsb", tag="xsb")
            if USE_FUSED_P3B:
                for ec in range(2):
                    x3 = psmall.tile([128, WIN], f32, name="x3", tag="x3")
                    xs_f = psmall.tile([128, 1], f32, name="xs_f", tag="xs_f")
                    # out = relu(ps_z) via op0=max; op1 is the REDUCTION op
                    # when accum_out is set: accum = sum(out)
                    nc.vector.tensor_scalar(
                        x3[:], ps_z[:, ec, :], 0.0, None,
                        op0=OP.max, op1=OP.add, accum_out=xs_f[:])
                    nc.gpsimd.tensor_copy(xsb[:, ec:ec + 1], xs_f[:])
            else:
                r1 = psmall.tile([128, 2, WIN], f32, name="r1", tag="r1")
                for ec in range(2):
                    nc.scalar.activation(r1[:, ec, :], ps_z[:, ec, :],
                                         AF.Relu, bias=b2col[:, ec:ec + 1])
                for ec in range(2):
                    x3 = psmall.tile([128, WIN], f32, name="x3", tag="x3")
                    xs_f = psmall.tile([128, 1], f32, name="xs_f", tag="xs_f")
                    nc.vector.scalar_tensor_tensor(
                        x3[:], r1[:, ec, :], 1.0,
                        mw[:, b * WIN:(b + 1) * WIN],
                        op0=OP.mult, op1=OP.mult, accum_out=xs_f[:])
                    nc.gpsimd.tensor_copy(xsb[:, ec:ec + 1], xs_f[:])
            T[b]["xsb"] = xsb

        def emit_p4l(b):
            # transposed logits: ps_lT[:, tc] = sum_dc axtb[:,dc,tc*128:].T @ xs
            at, ao = axtb_t[b]
            axtbs = at[:, ao:ao + 2 * S].rearrange("p (c s) -> p c s", c=2)
            xsb = T[b]["xsb"]
            lo = psLO.tile([128, 260], f32, name="ps_lo", tag="ps_lo")
            for tc_ in range(4):
                for dc in range(2):
                    nc.tensor.matmul(lo[:, tc_:tc_ + 1],
                                     axtbs[:, dc, ts(tc_, 128)],
                                     xsb[:, dc:dc + 1],
                                     start=(dc == 0), stop=(dc == 1))
            pexp = psmall.tile([128, 4], bf16, name="pexp", tag="pexp")
            nc.scalar.activation(pexp[:], lo[:, 0:4], AF.Exp,
                                 accum_out=sumP[:, b:b + 1])
            T[b]["pexp"] = pexp
            T[b]["lo"] = lo

        def emit_ws(b):
            # weighted sum: out[1, D] = sum_tc pexp[:, tc].T @ xnb[:, tc, :]
            at, ao = xnb_t[b]
            xns = at[:, ao:ao + 2 * S].rearrange("p (c s) -> p c s", c=4)
            pexp = T[b]["pexp"]
            lo = T[b]["lo"]
            for tc_ in range(4):
                nc.tensor.matmul(lo[0:1, 4:4 + D], pexp[:, tc_:tc_ + 1],
                                 xns[:, tc_, :],
                                 start=(tc_ == 0), stop=(tc_ == 3))
            outR = outRa if b < HB else outRb
            bo = b if b < HB else b - HB
            if b % 2 == 0:
                nc.scalar.copy(outR[:, bo * D:(bo + 1) * D], lo[0:1, 4:4 + D])
            else:
                nc.vector.tensor_copy(outR[:, bo * D:(bo + 1) * D],
                                      lo[0:1, 4:4 + D])

        for i in range(NSTEP):
            # dense block first (keeps the PE array busy so the HAM clock
            # gate stays at K=8/8), then the sparse tiny-matmul block, which
            # is well under the ~3.4us MID window that would re-throttle
            if i < bpc:
                emit_p1(i)
            if 0 <= i - LAG_P2 < bpc:
                emit_p2h(i - LAG_P2, 0)
                emit_p2h(i - LAG_P2, 1)
            if 0 <= i - LAG_WS < bpc:
                emit_ws(i - LAG_WS)
            if 0 <= i - LAG_P3 < bpc:
                emit_p3a(i - LAG_P3)
            if 0 <= i - LAG_P4 < bpc:
                emit_p4l(i - LAG_P4)
            if 0 <= i - LAG_P3 < bpc:
                emit_p3b(i - LAG_P3)
            if i - LAG_WS == HB - 1:
                # first-half output rows are final: overlap their store
                nc.sync.dma_start(outR_d[:, 0:HB * D], outRa[:])

        # sume[1, bpc] = ones.T @ sumP  (single f32 matmul)
        ps_s = psLO.tile([128, 260], f32, name="ps_sm", tag="ps_lo")
        nc.tensor.matmul(ps_s[0:1, 0:bpc], onescol[:], sumP[:])
        nc.scalar.copy(sume[:], ps_s[0:1, 0:bpc])

        nc.sync.dma_start(sume_d[:], sume[:])
        nc.sync.dma_start(outR_d[:, HB * D:], outRb[:])

    nc.compile()
    return nc


def _f8(x):
    return np.clip(x, -240.0, 240.0).astype(F8)


def _plan(inputs):
    """Host-side preprocessing: fold position weight / degree norm / fp8
    scales; sort samples by chunk count into per-core slots; pack per-core
    DRAM blobs. order[b*NCORES + c] is the original sample index placed in
    slot b of core c."""
    text_out = np.asarray(inputs["text_out"], dtype=np.float32)
    adj = np.asarray(inputs["adj"], dtype=np.float32)
    W1 = np.asarray(inputs["W1"], dtype=np.float32)
    b1 = np.asarray(inputs["b1"], dtype=np.float32)
    W2 = np.asarray(inputs["W2"], dtype=np.float32)
    b2 = np.asarray(inputs["b2"], dtype=np.float32)
    tl = np.asarray(inputs["text_len"]).astype(np.int64)
    al = np.asarray(inputs["aspect_len"]).astype(np.int64)
    ll = np.asarray(inputs["left_len"]).astype(np.int64)

    n_all = np.minimum(4, np.maximum(2, (tl + 127) // 128)).astype(np.int64)
    # descending: big-n slots first (denser warmup, lighter pipeline drain)
    order = np.argsort(-n_all, kind="stable")       # [B]
    n_slots = tuple(int(n_all[order[b * NCORES:(b + 1) * NCORES]].max())
                    for b in range(BPC))
    axt_off, adj_off, awm_off = _offsets(n_slots)

    j = np.arange(S)[None, :]
    start = ll[:, None]
    end = (ll + al - 1)[:, None]
    ctxlen = (tl - al).astype(np.float32)[:, None]
    w = np.where(j < start, 1.0 - (start - j) / ctxlen,
                 np.where(j <= end, 0.0,
                          np.where(j < tl[:, None], 1.0 - (j - end) / ctxlen,
                                   0.0))).astype(np.float32)      # [B,S]
    dinv = (1.0 / (adj.sum(axis=2) + 1.0)).astype(np.float32)     # [B,S]

    # transposed adjacency, position weight (t) and 1/den (s) folded, *4096
    adjTw = adj.transpose(0, 2, 1) * (4096.0 * w[:, :, None]) * dinv[:, None, :]
    adj8f = _f8(adjTw)                  # [B, t(S), s(S)]
    xT = text_out.transpose(0, 2, 1)    # [B, D, S]
    axt8f = _f8(xT)
    axtbf = xT.astype(BF)
    xnbf = text_out.astype(BF)          # [B, S(t), D]

    win = np.clip(ll[:, None] + np.arange(WIN)[None, :], 0, S - 1)  # [B,WIN]
    adj_win = np.take_along_axis(adj, win[:, :, None], axis=1)      # [B,WIN,S]
    dinvW = np.take_along_axis(dinv, win, axis=1)                   # [B,WIN]
    adjWTw = (adj_win.transpose(0, 2, 1) * (4096.0 * w[:, :, None])
              * dinvW[:, None, :])
    awm8f = _f8(adjWTw)                 # [B, s(S), WIN]

    # 0 on active window cols, -1000 on masked: relu applies the mask
    negmf = np.where(np.arange(WIN)[None, :] < al[:, None],
                     0.0, -1000.0).astype(BF)       # [B, WIN]

    W1s8 = _f8(np.ascontiguousarray(
        (16.0 * W1).reshape(2, 128, D).transpose(1, 0, 2)))
    W2bb = np.ascontiguousarray(
        W2.reshape(2, 128, D).transpose(1, 0, 2)).astype(BF)
    b1r16 = np.ascontiguousarray(
        (65536.0 * np.tile(b1, 2)).reshape(1, 2, D)).astype(BF)
    b2rr = np.ascontiguousarray(b2.reshape(1, 2, 128)).astype(BF)
    b1B8 = np.ascontiguousarray(np.broadcast_to(
        (256.0 * np.tile(b1, 2)).reshape(1, 2, D), (128, 2, D))).astype(BF)
    b2col = np.ascontiguousarray(b2.reshape(2, 128).T).astype(np.float32)

    in_maps = []
    for c in range(NCORES):
        idx = order[np.arange(BPC) * NCORES + c]   # slot b -> order[b*8+c]
        axt_p = np.empty((128, axt_off[-1]), dtype=F8)
        adj_p = np.empty((128, adj_off[-1]), dtype=F8)
        awm_p = np.empty((128, awm_off[-1]), dtype=F8)
        negm_p = np.empty((1, BPC * WIN), dtype=BF)
        mw_p = np.empty((128, BPC * WIN), dtype=BF)
        axtb_p = np.empty((128, BPC * 2 * S), dtype=BF)
        xnb_p = np.empty((128, BPC * 2 * S), dtype=BF)
        for b in range(BPC):
            bi = idx[b]
            n = n_slots[b]
            sa = 128 * n
            # axt8: [D, S] -> [128, 2(dc), 128n] -> flat
            axt_p[:, axt_off[b]:axt_off[b + 1]] = (
                axt8f[bi].reshape(2, 128, S)[:, :, :sa]
                .transpose(1, 0, 2).reshape(128, 2 * sa))
            # adj8: [t(S), s(S)] -> [128(t in chunk), n(tc), 128n(s)]
            adj_p[:, adj_off[b]:adj_off[b + 1]] = (
                adj8f[bi].reshape(4, 128, S)[:n, :, :sa]
                .transpose(1, 0, 2).reshape(128, n * sa))
            # awm: [s(S), WIN] -> [128, n(sc), WIN]
            awm_p[:, awm_off[b]:awm_off[b + 1]] = (
                awm8f[bi].reshape(4, 128, WIN)[:n]
                .transpose(1, 0, 2).reshape(128, n * WIN))
            negm_p[0, b * WIN:(b + 1) * WIN] = negmf[bi]
            mw_p[:, b * WIN:(b + 1) * WIN] = (negmf[bi] == 0).astype(BF)[None, :]
            # axtb: [D, S] -> [128, 2(dc), S]
            axtb_p[:, b * 2 * S:(b + 1) * 2 * S] = (
                axtbf[bi].reshape(2, 128, S)
                .transpose(1, 0, 2).reshape(128, 2 * S))
            # xnb: [S(t), D] -> [128(t in chunk), 4(tc), D]
            xnb_p[:, b * 2 * S:(b + 1) * 2 * S] = (
                xnbf[bi].reshape(4, 128, D)
                .transpose(1, 0, 2).reshape(128, 4 * D))
        in_maps.append({
            "axt8": np.ascontiguousarray(axt_p),
            "adj8": np.ascontiguousarray(adj_p),
            "awm": np.ascontiguousarray(awm_p),
            "negm": np.ascontiguousarray(negm_p),
            "axtb": np.ascontiguousarray(axtb_p),
            "xnb": np.ascontiguousarray(xnb_p),
            "W1s8": W1s8, "W2b": W2bb, "b1r16": b1r16, "b2r": b2rr,
            "b1B8": b1B8, "b2col": b2col, "mw": mw_p,
        })
    return in_maps, n_slots, order


def _assemble(results, order):
    out = np.empty((B, D), dtype=np.float32)
    for c in range(NCORES):
        outR = results[c]["outR"].reshape(BPC, D)
        sume = results[c]["sume"].reshape(-1)  # [BPC]
        for b in range(BPC):
            out[order[b * NCORES + c]] = outR[b] / sume[b]
    return out


def kernel(**inputs):
    from concourse.bass_utils import run_bass_kernel_spmd

    in_maps, n_slots, order = _plan(inputs)
    key = (BPC, n_slots)
    if key not in _nc_cache:
        _nc_cache[key] = _build_nc(BPC, n_slots)
    nc = _nc_cache[key]
    res = run_bass_kernel_spmd(nc, in_maps, list(range(NCORES)))
    return _assemble(res.results, order)


# revision 39
# speedup vs baseline: 1.2676x; 1.0420x over previous
"""ASGCN unit kernel for 8 Trainium2 NeuronCores (data-parallel over batch).

Contract: kernel(**inputs) takes the FULL unsharded inputs and returns the
FULL [128, 256] float32 output. Batch is sharded 16 samples/core across 8
cores; all parameters are replicated.

v2 design (evolved from the fp8 baseline after trace analysis):
  - position_weight, degree norm and fp8 scales are folded on host into the
    transposed adjacency (adjTw = adjT * 4096 * w[t] * dinv[s]) exactly as
    before; samples are sorted by n = ceil(text_len/128) into per-core slots
    sharing one slot->n pattern (SPMD) so matmuls skip structurally-zero
    128-chunks.
  - ALL inputs are shipped as a few large per-core packed DRAM blobs and
    loaded with ~18 big DMAs on the sync queue (the old per-sample DMA
    scheme kept the sync engine ~90% busy generating descriptors).
  - attention is restructured to be (almost) all-tensor:
      * logits computed TRANSPOSED: ps_lT[t,1] per 128-chunk via 8 tiny
        matmuls (lhsT = bf16 xT chunk, rhs = xs column),
      * exp on [128,4] (scalar engine, 128-partition utilization) with the
        per-partition accumulator collected into sumP[:, b],
      * weighted sum as 4 N=256 bf16 matmuls against a NORMAL-layout bf16
        copy of text_out (new input), giving the output row [1,256] in PSUM,
      * sum(exp) for all 16 samples reduced with ONE final f32 matmul
        (ones.T @ sumP).
    This removes the p-broadcast matmul, the [128,512] scalar copy and the
    two [128,512] vector accumulate-STTs per sample of the old design.
  - GCN layer 2 (window) runs fp8 for the adjacency contraction then bf16
    for the W2 matmul (better accuracy than the old all-fp8 path).
  - PSUM->SBUF epilogues are spread across scalar/vector; gpsimd (no PSUM
    port) takes the SBUF-only relu+cast work.
"""

import sys

if "/opt/trn_rl_repo" not in sys.path:
    sys.path.insert(0, "/opt/trn_rl_repo")

import numpy as np
import ml_dtypes

B, S, D, WIN = 128, 512, 256, 8
NCORES = 8
BPC = B // NCORES  # samples per core
BF = ml_dtypes.bfloat16
F8 = ml_dtypes.float8_e4m3  # TRN fp8e4: max +-240

_nc_cache = {}
USE_BIAS_MM = False
USE_FUSED_P3B = True


def _offsets(n_slots):
    """Per-slot element offsets (per partition) into the packed blobs."""
    axt_off, adj_off, awm_off = [0], [0], [0]
    for n in n_slots:
        axt_off.append(axt_off[-1] + 2 * 128 * n)      # [128, 2, 128n] fp8
        adj_off.append(adj_off[-1] + n * 128 * n)      # [128, n, 128n] fp8
        awm_off.append(awm_off[-1] + n * WIN)          # [128, n, WIN] fp8
    return axt_off, adj_off, awm_off


def _build_nc(bpc, n_slots):
    from contextlib import ExitStack

    import concourse.bass as bass
    import concourse.tile as tile
    from concourse import bacc, mybir

    dt = mybir.dt
    f32, bf16, f8 = dt.float32, dt.bfloat16, dt.float8e4
    AF = mybir.ActivationFunctionType
    OP = mybir.AluOpType
    DR = mybir.MatmulPerfMode.DoubleRow
    ts = bass.ts

    axt_off, adj_off, awm_off = _offsets(n_slots)

    nc = bacc.Bacc("TRN2", target_bir_lowering=False, debug=False,
                   num_devices=NCORES)

    # --- DRAM parameters: packed per-core blobs ---
    axt8_d = nc.declare_dram_parameter("axt8", [128, axt_off[-1]], f8,
                                       isOutput=False)
    adj8_d = nc.declare_dram_parameter("adj8", [128, adj_off[-1]], f8,
                                       isOutput=False)
    awm_d = nc.declare_dram_parameter("awm", [128, awm_off[-1]], f8,
                                      isOutput=False)
    negm_d = nc.declare_dram_parameter("negm", [1, bpc * WIN], bf16,
                                       isOutput=False)
    axtb_d = nc.declare_dram_parameter("axtb", [128, bpc * 2 * S], bf16,
                                       isOutput=False)
    xnb_d = nc.declare_dram_parameter("xnb", [128, bpc * 2 * S], bf16,
                                      isOutput=False)
    W1_d = nc.declare_dram_parameter("W1s8", [128, 2, D], f8, isOutput=False)
    W2_d = nc.declare_dram_parameter("W2b", [128, 2, D], bf16, isOutput=False)
    b1_d = nc.declare_dram_parameter("b1r16", [1, 2, D], bf16, isOutput=False)
    b2_d = nc.declare_dram_parameter("b2r", [1, 2, 128], bf16, isOutput=False)
    b1B_d = nc.declare_dram_parameter("b1B8", [128, 2, D], bf16, isOutput=False)
    b2c_d = nc.declare_dram_parameter("b2col", [128, 2], f32, isOutput=False)
    mw_d = nc.declare_dram_parameter("mw", [128, bpc * WIN], bf16, isOutput=False)
    outR_d = nc.declare_dram_parameter("outR", [1, bpc * D], f32,
                                       isOutput=True)
    sume_d = nc.declare_dram_parameter("sume", [1, bpc], f32, isOutput=True)

    LAG_P2, LAG_P3, LAG_P4, LAG_WS = 1, 2, 3, 4
    NSTEP = bpc + LAG_WS

    with tile.TileContext(nc) as tc, ExitStack() as ctx:
        const = ctx.enter_context(tc.tile_pool(name="const", bufs=1))
        pmid = ctx.enter_context(tc.tile_pool(name="pmid", bufs=6))
        psmall = ctx.enter_context(tc.tile_pool(name="psmall", bufs=8))
        pstage = ctx.enter_context(tc.tile_pool(name="pstage", bufs=1))
        psH = ctx.enter_context(tc.tile_pool(name="psH", bufs=2, space="PSUM"))
        psG = ctx.enter_context(tc.tile_pool(name="psG", bufs=2, space="PSUM"))
        psS = ctx.enter_context(tc.tile_pool(name="psS", bufs=1, space="PSUM"))
        psW = ctx.enter_context(tc.tile_pool(name="psW", bufs=1, space="PSUM"))
        # logits column [:, 0:4] and the ws output row [0:1, 4:260] share one
        # bank-sized tile (they are serially dependent through exp anyway)
        psLO = ctx.enter_context(tc.tile_pool(name="psLO", bufs=2,
                                              space="PSUM"))

        # ---- PE warmup: the HAM clock gate starts at K=4/8 (1.2 GHz) and
        # only opens after ~3.4us of sustained array activity. Burn dummy
        # dense DR matmuls during the initial DMA-wait window (tensor would
        # be idle anyway) so the real p1/p2 stream starts at 2.4 GHz. ----
        dumw = const.tile([128, 2, 128], f8, tag="dumw")
        nc.vector.memset(dumw[:], 1.0)
        ps_w = psW.tile([128, 128], f32, tag="ps_w")
        for _ in range(28):
            nc.tensor.matmul(ps_w[:], dumw[:, :, 0:128], dumw[:, :, 0:128],
                             perf_mode=DR)

        # ---- input SBUF blobs + the DMA schedule (sync queue only) ----
        W1s8 = const.tile([128, 2, D], f8, tag="W1s8")
        nc.sync.dma_start(W1s8[:], W1_d[:])

        # chunked blobs: independent tiles so readers only wait their chunk;
        # leading chunks are small so the pipeline starts ASAP
        AXT_CH = [(0, 2), (2, 6), (8, 8)]
        ADJ_CH = [(0, 2), (2, 2), (4, 4), (8, 4), (12, 4)]
        ABC, XNC = 4, 4  # slots per chunk
        axt_t, adj_t, axtb_t, xnb_t = {}, {}, {}, {}

        def dma_axt(c0, cnt):
            e0, e1 = axt_off[c0], axt_off[min(c0 + cnt, bpc)]
            t = const.tile([128, e1 - e0], f8, name=f"axt{c0}", tag=f"axt{c0}")
            nc.sync.dma_start(t[:], axt8_d[:, e0:e1])
            for b in range(c0, min(c0 + cnt, bpc)):
                axt_t[b] = (t, axt_off[b] - e0)

        def dma_adj(c0, cnt):
            e0, e1 = adj_off[c0], adj_off[min(c0 + cnt, bpc)]
            t = const.tile([128, e1 - e0], f8, name=f"adj{c0}", tag=f"adj{c0}")
            nc.sync.dma_start(t[:], adj8_d[:, e0:e1])
            for b in range(c0, min(c0 + cnt, bpc)):
                adj_t[b] = (t, adj_off[b] - e0)

        def dma_axtb(c0):
            e0, e1 = c0 * 2 * S, min(c0 + ABC, bpc) * 2 * S
            t = const.tile([128, e1 - e0], bf16, name=f"axb{c0}",
                           tag=f"axb{c0}")
            nc.sync.dma_start(t[:], axtb_d[:, e0:e1])
            for b in range(c0, min(c0 + ABC, bpc)):
                axtb_t[b] = (t, (b - c0) * 2 * S)

        def dma_xnb(c0):
            e0, e1 = c0 * 2 * S, min(c0 + XNC, bpc) * 2 * S
            t = const.tile([128, e1 - e0], bf16, name=f"xnb{c0}",
                           tag=f"xnb{c0}")
            nc.sync.dma_start(t[:], xnb_d[:, e0:e1])
            for b in range(c0, min(c0 + XNC, bpc)):
                xnb_t[b] = (t, (b - c0) * 2 * S)

        dma_axt(*AXT_CH[0])
        dma_adj(*ADJ_CH[0])
        b1r16 = const.tile([1, 2, D], bf16, tag="b1r16")
        nc.sync.dma_start(b1r16[:], b1_d[:])
        dma_axt(*AXT_CH[1])
        dma_adj(*ADJ_CH[1])
        W2b = const.tile([128, 2, D], bf16, tag="W2b")
        nc.sync.dma_start(W2b[:], W2_d[:])
        b2r = const.tile([1, 2, 128], bf16, tag="b2r")
        nc.sync.dma_start(b2r[:], b2_d[:])
        awm = const.tile([128, awm_off[-1]], f8, tag="awm")
        nc.sync.dma_start(awm[:], awm_d[:])
        negm = const.tile([1, bpc * WIN], bf16, tag="negm")
        nc.sync.dma_start(negm[:], negm_d[:])
        b1B8 = const.tile([128, 2, D], bf16, tag="b1B8")
        nc.sync.dma_start(b1B8[:], b1B_d[:])
        b2col = const.tile([128, 2], f32, tag="b2col")
        nc.sync.dma_start(b2col[:], b2c_d[:])
        mw = const.tile([128, bpc * WIN], bf16, tag="mw")
        nc.sync.dma_start(mw[:], mw_d[:])
        dma_adj(*ADJ_CH[2])
        dma_axtb(0)
        dma_axt(*AXT_CH[2])
        dma_adj(*ADJ_CH[3])
        dma_axtb(4)
        dma_xnb(0)
        dma_adj(*ADJ_CH[4])
        dma_axtb(8)
        dma_xnb(4)
        dma_axtb(12)
        dma_xnb(8)
        dma_xnb(12)

        onescol = const.tile([128, 1], f32, tag="onescol")
        nc.vector.memset(onescol[:], 1.0)
        onesrow = const.tile([1, 128], bf16, tag="onesrow")
        nc.vector.memset(onesrow[:], 1.0)
        sumP = pstage.tile([128, bpc], f32, tag="sumP")
        HB = bpc // 2
        outRa = pstage.tile([1, HB * D], f32, tag="outRa")
        outRb = pstage.tile([1, HB * D], f32, tag="outRb")
        sume = pstage.tile([1, bpc], f32, tag="sume")

        T = {b: {} for b in range(bpc)}
        npair_ = bpc // 2

        def _pair(j):
            if 0 <= j < npair_:
                return (2 * j, 2 * j + 1)
            return ()

        def emit_p1(b):
            # h1[s,e] = x[s,:] @ W1 ; lhsT = fp8 xT slice, rhs = 16*W1.
            # PSUM = 16*h1 -> fp8 copy (scalar/vector alternating).
            n = n_slots[b]
            at, ao = axt_t[b]
            axt = at[:, ao:ao + 2 * 128 * n].rearrange(
                "p (c s) -> p c s", c=2)
            h1s8 = pmid.tile([128, 4, D], f8, name="h1s8", tag="h1s8")
            for sc in range(n):
                ps_h = psH.tile([128, D], f32, name="ps_h", tag="ps_h")
                nc.tensor.matmul(ps_h[:], axt[:, :, ts(sc, 128)],
                                 W1s8[:, :, :], perf_mode=DR)
                if sc % 2 == 0:
                    nc.scalar.copy(h1s8[:, sc, :], ps_h[:])
                else:
                    nc.vector.tensor_copy(h1s8[:, sc, :], ps_h[:])
            T[b]["h1s8"] = h1s8

        def emit_p2h(b, half):
            # g1 = b1 + adjTw.T @ h1 ; x2 = fp8(relu(256*g1))
            n = n_slots[b]
            if half >= (n + 1) // 2:
                return
            at, ao = adj_t[b]
            adjs = at[:, ao:ao + n * 128 * n].rearrange(
                "p (c s) -> p c s", c=n)
            h1s8 = T[b]["h1s8"]
            if half == 0:
                x2 = pmid.tile([128, 4, D], f8, name="x2", tag="x2")
                T[b]["x2"] = x2
            else:
                x2 = T[b]["x2"]
            if True:
                w_ = min(2, n - 2 * half)
                ps_g = psG.tile([128, 2, D], f32, name="ps_g", tag="ps_g")
                if USE_BIAS_MM:
                    # bias first: PSUM = 65536*b1 via a K=1 bf16 matmul
                    nc.tensor.matmul(ps_g[:, 0:w_, :], onesrow[:],
                                     b1r16[:, 0:w_, :], start=True, stop=False)
                for sci in range(w_):
                    sc = 2 * half + sci
                    # DoubleRow over t-chunk pairs (fp8: 2 k-tiles/inst);
                    # each sci slice is its own open/close psum group
                    for tp in range(n // 2):
                        nc.tensor.matmul(
                            ps_g[:, sci, :],
                            adjs[:, 2 * tp:2 * tp + 2, ts(sc, 128)],
                            h1s8[:, 2 * tp:2 * tp + 2, :],
                            perf_mode=DR,
                            start=(not USE_BIAS_MM and tp == 0),
                            stop=(n % 2 == 0 and tp == n // 2 - 1))
                    if n % 2:
                        nc.tensor.matmul(
                            ps_g[:, sci, :],
                            adjs[:, n - 1, ts(sc, 128)],
                            h1s8[:, n - 1, :],
                            start=False,
                            stop=True)
                if USE_BIAS_MM:
                    # x2 = fp8(relu(2^-8 * PSUM)) straight out of PSUM
                    if half == 0:
                        nc.scalar.activation(
                            x2[:, 0:w_, :], ps_g[:, 0:w_, :], AF.Relu,
                            scale=1.0 / 256.0)
                    else:
                        nc.vector.tensor_scalar(
                            x2[:, 2:2 + w_, :], ps_g[:, 0:w_, :],
                            1.0 / 256.0, 0.0, op0=OP.mult, op1=OP.max)
                else:
                    gt = pmid.tile([128, 2, D], bf16, name="gt", tag="gt")
                    nc.vector.scalar_tensor_tensor(
                        gt[:, 0:w_, :], ps_g[:, 0:w_, :], 1.0 / 256.0,
                        b1B8[:, 0:w_, :], op0=OP.mult, op1=OP.add)
                    if half == 0:
                        nc.scalar.activation(
                            x2[:, 0:w_, :], gt[:, 0:w_, :], AF.Relu)
                    else:
                        nc.vector.tensor_scalar(
                            x2[:, 2:2 + w_, :], gt[:, 0:w_, :],
                            1.0, 0.0, op0=OP.mult, op1=OP.max)

        def emit_p3a(b):
            # window layer: ps_y = (256 x2).T @ (4096 awm) = 2^20 yT
            n = n_slots[b]
            x2 = T[b]["x2"]
            awms = awm[:, awm_off[b]:awm_off[b + 1]].rearrange(
                "p (c w) -> p c w", c=n)
            ps_y = psS.tile([128, 2, WIN], f32, name="ps_y", tag="ps_s")
            for dc in range(2):
                for sp in range(n // 2):
                    nc.tensor.matmul(ps_y[:, dc, :],
                                     x2[:, 2 * sp:2 * sp + 2, ts(dc, 128)],
                                     awms[:, 2 * sp:2 * sp + 2, :],
                                     perf_mode=DR,
                                     start=(sp == 0),
                                     stop=(n % 2 == 0 and sp == n // 2 - 1))
                if n % 2:
                    nc.tensor.matmul(ps_y[:, dc, :],
                                     x2[:, n - 1, ts(dc, 128)],
                                     awms[:, n - 1, :],
                                     start=False, stop=True)
            if b % 2 == 0:
                yTb2 = psmall.tile([128, 2, 2, WIN], bf16, name="yTb2",
                                   tag="yTb2")
                T[b]["yTb2"] = yTb2
            else:
                yTb2 = T[b - 1]["yTb2"]
            nc.vector.tensor_scalar(yTb2[:, :, b % 2, :], ps_y[:],
                                    2.0 ** -20, 0.0, op0=OP.mult, op1=OP.add)

        def emit_p3b2(b0):
            # paired (slots b0, b0+1): ps_z = W2b.T @ [yT_b0 | yT_b1] + b2
            # - 1000*mask ; xs[b] = sum_w relu(z_b) per slot
            yTb2 = T[b0]["yTb2"]
            ps_z = psS.tile([128, 2, 2, WIN], f32, name="ps_z", tag="ps_s")
            for ec in range(2):
                for dc in range(2):
                    nc.tensor.matmul(ps_z[:, ec, :, :],
                                     W2b[:, dc, ts(ec, 128)],
                                     yTb2[:, dc, :, :],
                                     start=(dc == 0), stop=False)
                # + b2[e] (rank-1: b2 slice as weights x ones-pair) and
                # -1000 on masked window cols: relu also applies the mask
                nc.tensor.matmul(ps_z[:, ec, :, :], b2r[:, ec, :],
                                 onesrow[0:1, 0:2 * WIN],
                                 start=False, stop=False)
                nc.tensor.matmul(ps_z[:, ec, :, :], onesrow[:],
                                 negm[:, b0 * WIN:(b0 + 2) * WIN],
                                 start=False, stop=True)
            for b in (b0, b0 + 1):
                xsb = psmall.tile([128, 2], bf16, name="xsb", tag="xsb")
                for ec in range(2):
                    x3 = psmall.tile([128, WIN], f32, name="x3", tag="x3")
                    xs_f = psmall.tile([128, 1], f32, name="xs_f", tag="xs_f")
                    # out = relu(ps_z) via op0=max; op1 is the REDUCTION op
                    # when accum_out is set: accum = sum(out)
                    nc.vector.tensor_scalar(
                        x3[:], ps_z[:, ec, b - b0, :], 0.0, None,
                        op0=OP.max, op1=OP.add, accum_out=xs_f[:])
                    nc.gpsimd.tensor_copy(xsb[:, ec:ec + 1], xs_f[:])
                T[b]["xsb"] = xsb

        def emit_p4l(b):
            # transposed logits: ps_lT[:, tc] = sum_dc axtb[:,dc,tc*128:].T @ xs
            at, ao = axtb_t[b]
            axtbs = at[:, ao:ao + 2 * S].rearrange("p (c s) -> p c s", c=2)
            xsb = T[b]["xsb"]
            lo = psLO.tile([128, 260], f32, name="ps_lo", tag="ps_lo")
            for tc_ in range(4):
                for dc in range(2):
                    nc.tensor.matmul(lo[:, tc_:tc_ + 1],
                                     axtbs[:, dc, ts(tc_, 128)],
                                     xsb[:, dc:dc + 1],
                                     start=(dc == 0), stop=(dc == 1))
            pexp = psmall.tile([128, 4], bf16, name="pexp", tag="pexp")
            nc.scalar.activation(pexp[:], lo[:, 0:4], AF.Exp,
                                 accum_out=sumP[:, b:b + 1])
            T[b]["pexp"] = pexp
            T[b]["lo"] = lo

        def emit_ws(b):
            # weighted sum: out[1, D] = sum_tc pexp[:, tc].T @ xnb[:, tc, :]
            at, ao = xnb_t[b]
            xns = at[:, ao:ao + 2 * S].rearrange("p (c s) -> p c s", c=4)
            pexp = T[b]["pexp"]
            lo = T[b]["lo"]
            for tc_ in range(4):
                nc.tensor.matmul(lo[0:1, 4:4 + D], pexp[:, tc_:tc_ + 1],
                                 xns[:, tc_, :],
                                 start=(tc_ == 0), stop=(tc_ == 3))
            outR = outRa if b < HB else outRb
            bo = b if b < HB else b - HB
            if b % 2 == 0:
                nc.scalar.copy(outR[:, bo * D:(bo + 1) * D], lo[0:1, 4:4 + D])
            else:
                nc.vector.tensor_copy(outR[:, bo * D:(bo + 1) * D],
                                      lo[0:1, 4:4 + D])

        def keepwarm():
            nc.tensor.matmul(ps_w[:], dumw[:, :, 0:128], dumw[:, :, 0:128],
                             perf_mode=DR)

        npair = bpc // 2
        for i in range(npair + LAG_WS):
            # dense block first (keeps the PE array busy so the HAM clock
            # gate stays at K=8/8), then the sparse tiny-matmul block broken
            # up with dummy dense matmuls so no ~3.4us HAM window goes idle
            for b in _pair(i):
                emit_p1(b)
            for b in _pair(i - LAG_P2):
                emit_p2h(b, 0)
                emit_p2h(b, 1)
            for b in _pair(i - LAG_WS):
                emit_ws(b)
            for b in _pair(i - LAG_P3):
                emit_p3a(b)
            keepwarm()
            for b in _pair(i - LAG_P4):
                emit_p4l(b)
            keepwarm()
            if 0 <= i - LAG_P3 < npair:
                emit_p3b2(2 * (i - LAG_P3))
            if i - LAG_WS == HB // 2 - 1:
                # first-half output rows are final: overlap their store
                nc.sync.dma_start(outR_d[:, 0:HB * D], outRa[:])

        # sume[1, bpc] = ones.T @ sumP  (single f32 matmul)
        ps_s = psLO.tile([128, 260], f32, name="ps_sm", tag="ps_lo")
        nc.tensor.matmul(ps_s[0:1, 0:bpc], onescol[:], sumP[:])
        nc.scalar.copy(sume[:], ps_s[0:1, 0:bpc])

        nc.sync.dma_start(sume_d[:], sume[:])
        nc.sync.dma_start(outR_d[:, HB * D:], outRb[:])

    nc.compile()
    return nc


def _f8(x):
    return np.clip(x, -240.0, 240.0).astype(F8)


def _plan(inputs):
    """Host-side preprocessing: fold position weight / degree norm / fp8
    scales; sort samples by chunk count into per-core slots; pack per-core
    DRAM blobs. order[b*NCORES + c] is the original sample index placed in
    slot b of core c."""
    text_out = np.asarray(inputs["text_out"], dtype=np.float32)
    adj = np.asarray(inputs["adj"], dtype=np.float32)
    W1 = np.asarray(inputs["W1"], dtype=np.float32)
    b1 = np.asarray(inputs["b1"], dtype=np.float32)
    W2 = np.asarray(inputs["W2"], dtype=np.float32)
    b2 = np.asarray(inputs["b2"], dtype=np.float32)
    tl = np.asarray(inputs["text_len"]).astype(np.int64)
    al = np.asarray(inputs["aspect_len"]).astype(np.int64)
    ll = np.asarray(inputs["left_len"]).astype(np.int64)

    n_all = np.minimum(4, np.maximum(2, (tl + 127) // 128)).astype(np.int64)
    # descending: big-n slots first (denser warmup, lighter pipeline drain)
    order = np.argsort(-n_all, kind="stable")       # [B]
    n_slots = tuple(int(n_all[order[b * NCORES:(b + 1) * NCORES]].max())
                    for b in range(BPC))
    axt_off, adj_off, awm_off = _offsets(n_slots)

    j = np.arange(S)[None, :]
    start = ll[:, None]
    end = (ll + al - 1)[:, None]
    ctxlen = (tl - al).astype(np.float32)[:, None]
    w = np.where(j < start, 1.0 - (start - j) / ctxlen,
                 np.where(j <= end, 0.0,
                          np.where(j < tl[:, None], 1.0 - (j - end) / ctxlen,
                                   0.0))).astype(np.float32)      # [B,S]
    dinv = (1.0 / (adj.sum(axis=2) + 1.0)).astype(np.float32)     # [B,S]

    # transposed adjacency, position weight (t) and 1/den (s) folded, *4096
    adjTw = adj.transpose(0, 2, 1) * (4096.0 * w[:, :, None]) * dinv[:, None, :]
    adj8f = _f8(adjTw)                  # [B, t(S), s(S)]
    xT = text_out.transpose(0, 2, 1)    # [B, D, S]
    axt8f = _f8(xT)
    axtbf = xT.astype(BF)
    xnbf = text_out.astype(BF)          # [B, S(t), D]

    win = np.clip(ll[:, None] + np.arange(WIN)[None, :], 0, S - 1)  # [B,WIN]
    adj_win = np.take_along_axis(adj, win[:, :, None], axis=1)      # [B,WIN,S]
    dinvW = np.take_along_axis(dinv, win, axis=1)                   # [B,WIN]
    adjWTw = (adj_win.transpose(0, 2, 1) * (4096.0 * w[:, :, None])
              * dinvW[:, None, :])
    awm8f = _f8(adjWTw)                 # [B, s(S), WIN]

    # 0 on active window cols, -1000 on masked: relu applies the mask
    negmf = np.where(np.arange(WIN)[None, :] < al[:, None],
                     0.0, -1000.0).astype(BF)       # [B, WIN]

    W1s8 = _f8(np.ascontiguousarray(
        (16.0 * W1).reshape(2, 128, D).transpose(1, 0, 2)))
    W2bb = np.ascontiguousarray(
        W2.reshape(2, 128, D).transpose(1, 0, 2)).astype(BF)
    b1r16 = np.ascontiguousarray(
        (65536.0 * np.tile(b1, 2)).reshape(1, 2, D)).astype(BF)
    b2rr = np.ascontiguousarray(b2.reshape(1, 2, 128)).astype(BF)
    b1B8 = np.ascontiguousarray(np.broadcast_to(
        (256.0 * np.tile(b1, 2)).reshape(1, 2, D), (128, 2, D))).astype(BF)
    b2col = np.ascontiguousarray(b2.reshape(2, 128).T).astype(np.float32)

    in_maps = []
    for c in range(NCORES):
        idx = order[np.arange(BPC) * NCORES + c]   # slot b -> order[b*8+c]
        axt_p = np.empty((128, axt_off[-1]), dtype=F8)
        adj_p = np.empty((128, adj_off[-1]), dtype=F8)
        awm_p = np.empty((128, awm_off[-1]), dtype=F8)
        negm_p = np.empty((1, BPC * WIN), dtype=BF)
        mw_p = np.empty((128, BPC * WIN), dtype=BF)
        axtb_p = np.empty((128, BPC * 2 * S), dtype=BF)
        xnb_p = np.empty((128, BPC * 2 * S), dtype=BF)
        for b in range(BPC):
            bi = idx[b]
            n = n_slots[b]
            sa = 128 * n
            # axt8: [D, S] -> [128, 2(dc), 128n] -> flat
            axt_p[:, axt_off[b]:axt_off[b + 1]] = (
                axt8f[bi].reshape(2, 128, S)[:, :, :sa]
                .transpose(1, 0, 2).reshape(128, 2 * sa))
            # adj8: [t(S), s(S)] -> [128(t in chunk), n(tc), 128n(s)]
            adj_p[:, adj_off[b]:adj_off[b + 1]] = (
                adj8f[bi].reshape(4, 128, S)[:n, :, :sa]
                .transpose(1, 0, 2).reshape(128, n * sa))
            # awm: [s(S), WIN] -> [128, n(sc), WIN]
            awm_p[:, awm_off[b]:awm_off[b + 1]] = (
                awm8f[bi].reshape(4, 128, WIN)[:n]
                .transpose(1, 0, 2).reshape(128, n * WIN))
            negm_p[0, b * WIN:(b + 1) * WIN] = negmf[bi]
            mw_p[:, b * WIN:(b + 1) * WIN] = (negmf[bi] == 0).astype(BF)[None, :]
            # axtb: [D, S] -> [128, 2(dc), S]
            axtb_p[:, b * 2 * S:(b + 1) * 2 * S] = (
                axtbf[bi].reshape(2, 128, S)
                .transpose(1, 0, 2).reshape(128, 2 * S))
            # xnb: [S(t), D] -> [128(t in chunk), 4(tc), D]
            xnb_p[:, b * 2 * S:(b + 1) * 2 * S] = (
                xnbf[bi].reshape(4, 128, D)
                .transpose(1, 0, 2).reshape(128, 4 * D))
        in_maps.append({
            "axt8": np.ascontiguousarray(axt_p),
            "adj8": np.ascontiguousarray(adj_p),
            "awm": np.ascontiguousarray(awm_p),
            "negm": np.ascontiguousarray(negm_p),
            "axtb": np.ascontiguousarray(axtb_p),
            "xnb": np.ascontiguousarray(xnb_p),
            "W1s8": W1s8, "W2b": W2bb, "b1r16": b1r16, "b2r": b2rr,
            "b1B8": b1B8, "b2col": b2col, "mw": mw_p,
        })
    return in_maps, n_slots, order


def _assemble(results, order):
    out = np.empty((B, D), dtype=np.float32)
    for c in range(NCORES):
        outR = results[c]["outR"].reshape(BPC, D)
        sume = results[c]["sume"].reshape(-1)  # [BPC]
        for b in range(BPC):
            out[order[b * NCORES + c]] = outR[b] / sume[b]
    return out


def kernel(**inputs):
    from concourse.bass_utils import run_bass_kernel_spmd

    in_maps, n_slots, order = _plan(inputs)
    key = (BPC, n_slots)
    if key not in _nc_cache:
        _nc_cache[key] = _build_nc(BPC, n_slots)
    nc = _nc_cache[key]
    res = run_bass_kernel_spmd(nc, in_maps, list(range(NCORES)))
    return _assemble(res.results, order)
